# revision 10
# baseline (speedup 1.0000x reference)
"""Trainium2 Bass kernel for nn_MultiHeadAttention (B=2, S=2048, D=1024, H=16).

Sharding: batch*heads across 8 cores -> each core handles one batch element's
4 heads (core c: b = c//4, heads h0 = (c%4)*4 .. h0+4).

Per-core device program (all matmuls float32r, full-rate on PE):
  1. QKV projection from pre-transposed activations xT [1024, 2048]:
       Qt/Kt produced transposed ([q_dim, s], 2 heads stacked per 128
       partitions), V produced natural ([s, v_dim], 4 heads side by side)
       with an extra ones column (row-sum trick) appended per head.
  2. Attention per head in "scoresT" layout [key, query]: PE computes
       scoresT tiles, ScalarE applies additive mask + exp in one op
       (bias is per-partition = per-key), PE contracts exp-scores with
       Vaug -> unnormalized ctxT [65, q] where row 64 is the softmax sum.
  3. Normalize: reciprocal of sums, broadcast across partitions via a K=1
       ones-outer-product matmul, multiply on VectorE.
  4. Output projection: ctxT pairs (128 head-dims) x W_out rows -> partial
       output [2048, 1024], DMA'd straight from PSUM to DRAM.
Host sums the 4 partial outputs per batch element and adds b_out.
"""

import math

import numpy as np

N_HEADS = 16
DIM = 1024
DIM_PER_HEAD = 64
B = 2
S = 2048
SCALE = math.sqrt(DIM_PER_HEAD)
N_CORES = 8
HEADS_PER_CORE = 4

_cache = {}


def _build_program():
    import concourse.bass as bass
    import concourse.tile as tile
    from concourse import bacc, mybir

    f32 = mybir.dt.float32
    f32r = mybir.dt.float32r
    Exp = mybir.ActivationFunctionType.Exp

    nc = bacc.Bacc("TRN2", target_bir_lowering=False, debug=False,
                   num_devices=N_CORES)

    xT = nc.dram_tensor("xT", [DIM, S], f32r, kind="ExternalInput").ap()
    Wq = nc.dram_tensor("Wq", [DIM, 256], f32r, kind="ExternalInput").ap()
    Wk = nc.dram_tensor("Wk", [DIM, 256], f32r, kind="ExternalInput").ap()
    Wv = nc.dram_tensor("Wv", [DIM, 256], f32r, kind="ExternalInput").ap()
    Wo = nc.dram_tensor("Wo", [256, DIM], f32r, kind="ExternalInput").ap()
    bqk = nc.dram_tensor("bqk", [128, 4], f32, kind="ExternalInput").ap()
    bv = nc.dram_tensor("bv", [1, 256], f32r, kind="ExternalInput").ap()
    maskT = nc.dram_tensor("maskT", [128, 64], f32, kind="ExternalInput").ap()
    ones_d = nc.dram_tensor("ones_d", [128, 128], f32r, kind="ExternalInput").ap()
    out_d = nc.dram_tensor("out", [S, DIM], f32, kind="ExternalOutput").ap()

    with tile.TileContext(nc) as tc:
        with tc.tile_pool(name="const", bufs=1) as cpool, \
             tc.tile_pool(name="wpool", bufs=1) as wpool, \
             tc.tile_pool(name="qkv", bufs=1) as qkvp, \
             tc.tile_pool(name="xsub", bufs=10) as xsub, \
             tc.tile_pool(name="expp", bufs=3) as expp, \
             tc.tile_pool(name="ctxu", bufs=2) as ctxu, \
             tc.tile_pool(name="outsb", bufs=2) as outsb, \
             tc.tile_pool(name="rscr", bufs=1) as rscr, \
             tc.tile_pool(name="ps", bufs=2, space="PSUM") as ps:

            ones = cpool.tile([1, 128], f32r)
            nc.sync.dma_start(ones[:], ones_d[0:1, :])
            bqk_sb = cpool.tile([128, 4], f32)
            nc.sync.dma_start(bqk_sb[:], bqk[:])
            bv_sb = cpool.tile([1, 256], f32r)
            nc.sync.dma_start(bv_sb[:], bv[:])
            maskT_sb = cpool.tile([128, 64], f32)
            nc.sync.dma_start(maskT_sb[:], maskT[:])

            Wq_sb = wpool.tile([128, 8, 256], f32r)
            nc.sync.dma_start(Wq_sb[:], Wq.rearrange("(c p) j -> p c j", p=128))
            Wk_sb = wpool.tile([128, 8, 256], f32r)
            nc.sync.dma_start(Wk_sb[:], Wk.rearrange("(c p) j -> p c j", p=128))
            Wv_sb = wpool.tile([128, 8, 256], f32r)
            nc.sync.dma_start(Wv_sb[:], Wv.rearrange("(c p) j -> p c j", p=128))
            Wo_sb = wpool.tile([128, 2, 1024], f32r)
            nc.sync.dma_start(Wo_sb[:], Wo.rearrange("(c p) e -> p c e", p=128))

            Qt_sb = qkvp.tile([128, 2, S], f32r)
            Kt_sb = qkvp.tile([128, 2, S], f32r)
            Vaug = qkvp.tile([128, 4 * 16 * 65], f32r)
            v4 = Vaug.rearrange("p (h k j) -> p h k j", h=4, k=16)
            nc.sync.dma_start(v4[:, :, :, 64],
                              ones_d[:, 0:64].rearrange("p (h k) -> p h k", h=4))
            ctxT_norm = qkvp.tile([128, 2, S], f32r)

            # ---- QKV projection ----
            for sc in range(4):
                xts = []
                for dc in range(8):
                    xt_t = xsub.tile([128, 512], f32r, tag="x")
                    nc.sync.dma_start(
                        xt_t[:], xT[dc * 128:(dc + 1) * 128, sc * 512:(sc + 1) * 512])
                    xts.append(xt_t)
                for qk in range(2):
                    W = (Wq_sb, Wk_sb)[qk]
                    dest = (Qt_sb, Kt_sb)[qk]
                    for p in range(2):
                        ps_t = ps.tile([128, 512], f32, tag="a")
                        for dc in range(8):
                            nc.tensor.matmul(
                                ps_t[:], lhsT=W[:, dc, p * 128:(p + 1) * 128],
                                rhs=xts[dc][:], start=(dc == 0), stop=(dc == 7))
                        nc.vector.tensor_scalar_add(
                            dest[:, p, sc * 512:(sc + 1) * 512], ps_t[:],
                            bqk_sb[:, 2 * qk + p: 2 * qk + p + 1])
                for stl in range(4):
                    st = sc * 4 + stl
                    pv_t = ps.tile([128, 256], f32, tag="a")
                    for dc in range(8):
                        nc.tensor.matmul(
                            pv_t[:], lhsT=xts[dc][:, stl * 128:(stl + 1) * 128],
                            rhs=Wv_sb[:, dc, :], start=(dc == 0), stop=False)
                    nc.tensor.matmul(pv_t[:], lhsT=ones[:], rhs=bv_sb[:],
                                     start=False, stop=True)
                    nc.vector.tensor_copy(
                        v4[:, :, st, 0:64],
                        pv_t[:].rearrange("p (h j) -> p h j", h=4))

            # ---- attention per head ----
            for p in range(2):
                for hp in range(2):
                    h = p * 2 + hp
                    ctx_ps = ps.tile([65, S], f32, tag="ctx", bufs=1)
                    for half in range(2):
                        for kt in range(16):
                            sc_t = ps.tile([128, 1024], f32, tag="a")
                            for qc in range(2):
                                q0 = half * 1024 + qc * 512
                                nc.tensor.matmul(
                                    sc_t[:, qc * 512:(qc + 1) * 512],
                                    lhsT=Kt_sb[hp * 64:(hp + 1) * 64, p,
                                               kt * 128:(kt + 1) * 128],
                                    rhs=Qt_sb[hp * 64:(hp + 1) * 64, p, q0:q0 + 512],
                                    start=True, stop=True)
                            et = expp.tile([128, 1024], f32r, tag="et")
                            nc.scalar.activation(
                                et[:], sc_t[:], Exp,
                                bias=maskT_sb[:, kt * 4 + h: kt * 4 + h + 1],
                                scale=1.0)
                            for qc in range(2):
                                q0 = half * 1024 + qc * 512
                                nc.tensor.matmul(
                                    ctx_ps[:, q0:q0 + 512], lhsT=v4[:, h, kt, :],
                                    rhs=et[:, qc * 512:(qc + 1) * 512],
                                    start=(kt == 0), stop=(kt == 15))
                    # head epilogue: move ctx out of PSUM, normalize
                    ctxU = ctxu.tile([65, S], f32, tag="cu")
                    nc.vector.tensor_copy(ctxU[:], ctx_ps[:])
                    sums_t = rscr.tile([1, S], f32, tag="sm")
                    nc.sync.dma_start(sums_t[:], ctxU[64:65, :])
                    rs_t = rscr.tile([1, S], f32r, tag="rs")
                    with nc.allow_low_precision(reason="f32r rounding for matmul rhs"):
                        nc.vector.reciprocal(rs_t[:], sums_t[:])
                    if hp == 1:
                        ctxN = ctxu.tile([64, S], f32r, tag="cn", bufs=1)
                    for qc in range(4):
                        pb_t = ps.tile([128, 512], f32, tag="a")
                        nc.tensor.matmul(pb_t[:], lhsT=ones[:],
                                         rhs=rs_t[0:1, qc * 512:(qc + 1) * 512],
                                         start=True, stop=True)
                        if hp == 0:
                            tt_out = ctxT_norm[0:64, p, qc * 512:(qc + 1) * 512]
                        else:
                            tt_out = ctxN[0:64, qc * 512:(qc + 1) * 512]
                        nc.vector.tensor_mul(
                            tt_out,
                            ctxU[0:64, qc * 512:(qc + 1) * 512],
                            pb_t[0:64, :])
                    if hp == 1:
                        nc.sync.dma_start(ctxT_norm[64:128, p, :], ctxN[0:64, :])

            # ---- output projection ----
            for qt in range(16):
                po = ps.tile([128, 1024], f32, tag="a")
                for p in range(2):
                    for ec in range(2):
                        nc.tensor.matmul(
                            po[:, ec * 512:(ec + 1) * 512],
                            lhsT=ctxT_norm[:, p, qt * 128:(qt + 1) * 128],
                            rhs=Wo_sb[:, p, ec * 512:(ec + 1) * 512],
                            start=(p == 0), stop=(p == 1))
                ob = outsb.tile([128, 1024], f32, tag="ob")
                nc.vector.tensor_copy(ob[:], po[:])
                nc.sync.dma_start(out_d[qt * 128:(qt + 1) * 128, :], ob[:])

    nc.compile()
    return nc


def get_program():
    if "nc" not in _cache:
        _cache["nc"] = _build_program()
    return _cache["nc"]


def make_in_maps(query, mask, W_qkv, b_qkv, W_out, b_out):
    query = np.asarray(query, dtype=np.float32)
    mask = np.asarray(mask)
    W_qkv = np.asarray(W_qkv, dtype=np.float32)
    b_qkv = np.asarray(b_qkv, dtype=np.float32)
    W_out = np.asarray(W_out, dtype=np.float32)

    W3 = W_qkv.reshape(DIM, N_HEADS, DIM_PER_HEAD, 3)
    b3 = b_qkv.reshape(N_HEADS, DIM_PER_HEAD, 3)
    maskadd = np.where(mask[:, 0, :], np.float32(-30000.0), np.float32(0.0))

    in_maps = []
    for c in range(N_CORES):
        b = c // 4
        h0 = (c % 4) * HEADS_PER_CORE
        hs = slice(h0, h0 + HEADS_PER_CORE)
        Wq_c = np.ascontiguousarray(
            W3[:, hs, :, 0].reshape(DIM, 256) / SCALE)
        Wk_c = np.ascontiguousarray(W3[:, hs, :, 1].reshape(DIM, 256))
        Wv_c = np.ascontiguousarray(W3[:, hs, :, 2].reshape(DIM, 256))
        bq_c = (b3[hs, :, 0].reshape(256) / SCALE).astype(np.float32)
        bk_c = b3[hs, :, 1].reshape(256).astype(np.float32)
        bv_c = b3[hs, :, 2].reshape(1, 256).astype(np.float32)
        bqk_c = np.ascontiguousarray(
            np.stack([bq_c[:128], bq_c[128:], bk_c[:128], bk_c[128:]], axis=1))
        Wo_c = np.ascontiguousarray(W_out[h0 * 64:(h0 + 4) * 64, :])
        xT_c = np.ascontiguousarray(query[b].T)
        ma_c = maskadd[b * N_HEADS + h0: b * N_HEADS + h0 + 4]  # [4, 2048]
        maskT_c = np.ascontiguousarray(
            ma_c.reshape(4, 16, 128).transpose(2, 1, 0).reshape(128, 64)
        ).astype(np.float32)
        in_maps.append({
            "xT": xT_c, "Wq": Wq_c, "Wk": Wk_c, "Wv": Wv_c, "Wo": Wo_c,
            "bqk": bqk_c, "bv": bv_c, "maskT": maskT_c,
            "ones_d": np.ones((128, 128), dtype=np.float32),
        })
    return in_maps


def gather_outputs(results, b_out):
    b_out = np.asarray(b_out, dtype=np.float32)
    out = np.zeros((B, S, DIM), dtype=np.float32)
    for c in range(N_CORES):
        out[c // 4] += results[c]["out"]
    out += b_out[None, None, :]
    return out


def kernel(query, mask, W_qkv, b_qkv, W_out, b_out):
    from concourse.bass_utils import run_bass_kernel_spmd

    nc = get_program()
    in_maps = make_in_maps(query, mask, W_qkv, b_qkv, W_out, b_out)
    res = run_bass_kernel_spmd(nc, in_maps, list(range(N_CORES)))
    return gather_outputs(res.results, b_out)


# revision 12
# speedup vs baseline: 1.1855x; 1.1855x over previous
"""Trainium2 Bass kernel for nn_MultiHeadAttention (B=2, S=2048, D=1024, H=16).

Sharding: batch*heads across 8 cores -> each core handles one batch element's
4 heads (core c: b = c//4, heads h0 = (c%4)*4 .. h0+4).

Per-core device program (all matmuls float32r, full-rate on PE):
  1. QKV projection from pre-transposed activations xT [1024, 2048]:
       Qt/Kt produced transposed ([q_dim, s], 2 heads stacked per 128
       partitions), V produced natural ([s, v_dim], 4 heads side by side)
       with an extra ones column (row-sum trick) appended per head.
  2. Attention per head in "scoresT" layout [key, query]: PE computes
       scoresT tiles, ScalarE applies additive mask + exp in one op
       (bias is per-partition = per-key), PE contracts exp-scores with
       Vaug -> unnormalized ctxT [65, q] where row 64 is the softmax sum.
  3. Normalize: reciprocal of sums, broadcast across partitions via a K=1
       ones-outer-product matmul, multiply on VectorE.
  4. Output projection: ctxT pairs (128 head-dims) x W_out rows -> partial
       output [2048, 1024], DMA'd straight from PSUM to DRAM.
Host sums the 4 partial outputs per batch element and adds b_out.
"""

import math

import numpy as np

N_HEADS = 16
DIM = 1024
DIM_PER_HEAD = 64
B = 2
S = 2048
SCALE = math.sqrt(DIM_PER_HEAD)
N_CORES = 8
HEADS_PER_CORE = 4

_cache = {}


def _build_program():
    import concourse.bass as bass
    import concourse.tile as tile
    from concourse import bacc, mybir

    f32 = mybir.dt.float32
    f32r = mybir.dt.float32r
    bf16 = mybir.dt.float16
    Exp = mybir.ActivationFunctionType.Exp

    nc = bacc.Bacc("TRN2", target_bir_lowering=False, debug=False,
                   num_devices=N_CORES)

    xT = nc.dram_tensor("xT", [DIM, S], bf16, kind="ExternalInput").ap()
    Wq = nc.dram_tensor("Wq", [DIM, 256], bf16, kind="ExternalInput").ap()
    Wk = nc.dram_tensor("Wk", [DIM, 256], bf16, kind="ExternalInput").ap()
    Wv = nc.dram_tensor("Wv", [DIM, 256], bf16, kind="ExternalInput").ap()
    Wo = nc.dram_tensor("Wo", [256, DIM], f32r, kind="ExternalInput").ap()
    bqk = nc.dram_tensor("bqk", [128, 4], f32, kind="ExternalInput").ap()
    bv = nc.dram_tensor("bv", [1, 256], f32r, kind="ExternalInput").ap()
    maskT = nc.dram_tensor("maskT", [128, 64], f32, kind="ExternalInput").ap()
    ones_d = nc.dram_tensor("ones_d", [1, 128], f32r, kind="ExternalInput").ap()
    ones_bf = nc.dram_tensor("ones_bf", [128, 64], bf16, kind="ExternalInput").ap()
    out_d = nc.dram_tensor("out", [S, DIM], f32, kind="ExternalOutput").ap()

    with tile.TileContext(nc) as tc:
        with tc.tile_pool(name="const", bufs=1) as cpool, \
             tc.tile_pool(name="wpool", bufs=1) as wpool, \
             tc.tile_pool(name="qkv", bufs=1) as qkvp, \
             tc.tile_pool(name="xsub", bufs=10) as xsub, \
             tc.tile_pool(name="expp", bufs=3) as expp, \
             tc.tile_pool(name="ctxu", bufs=2) as ctxu, \
             tc.tile_pool(name="outsb", bufs=2) as outsb, \
             tc.tile_pool(name="rscr", bufs=1) as rscr, \
             tc.tile_pool(name="ps", bufs=2, space="PSUM") as ps:

            ones = cpool.tile([1, 128], f32r)
            nc.sync.dma_start(ones[:], ones_d[:])
            bqk_sb = cpool.tile([128, 4], f32)
            nc.sync.dma_start(bqk_sb[:], bqk[:])
            bv_sb = cpool.tile([1, 256], f32r)
            nc.sync.dma_start(bv_sb[:], bv[:])
            maskT_sb = cpool.tile([128, 64], f32)
            nc.sync.dma_start(maskT_sb[:], maskT[:])

            Wq_sb = wpool.tile([128, 8, 256], bf16)
            nc.sync.dma_start(Wq_sb[:], Wq.rearrange("(c p) j -> p c j", p=128))
            Wk_sb = wpool.tile([128, 8, 256], bf16)
            nc.sync.dma_start(Wk_sb[:], Wk.rearrange("(c p) j -> p c j", p=128))
            Wv_sb = wpool.tile([128, 8, 256], bf16)
            nc.sync.dma_start(Wv_sb[:], Wv.rearrange("(c p) j -> p c j", p=128))
            Wo_sb = wpool.tile([128, 2, 1024], f32r)
            nc.sync.dma_start(Wo_sb[:], Wo.rearrange("(c p) e -> p c e", p=128))

            Qt_sb = qkvp.tile([128, 2, S], bf16)
            Kt_sb = qkvp.tile([128, 2, S], bf16)
            Vaug = qkvp.tile([128, 4 * 16 * 65], bf16)
            v4 = Vaug.rearrange("p (h k j) -> p h k j", h=4, k=16)
            nc.sync.dma_start(v4[:, :, :, 64],
                              ones_bf[:, :].rearrange("p (h k) -> p h k", h=4))
            ctxT_norm = qkvp.tile([128, 2, S], f32r)

            # ---- QKV projection ----
            for sc in range(4):
                xts = []
                for dc in range(8):
                    xt_t = xsub.tile([128, 512], bf16, tag="x")
                    nc.sync.dma_start(
                        xt_t[:], xT[dc * 128:(dc + 1) * 128, sc * 512:(sc + 1) * 512])
                    xts.append(xt_t)
                for qk in range(2):
                    W = (Wq_sb, Wk_sb)[qk]
                    dest = (Qt_sb, Kt_sb)[qk]
                    for p in range(2):
                        ps_t = ps.tile([128, 512], f32, tag="a")
                        for dc in range(8):
                            nc.tensor.matmul(
                                ps_t[:], lhsT=W[:, dc, p * 128:(p + 1) * 128],
                                rhs=xts[dc][:], start=(dc == 0), stop=(dc == 7))
                        nc.vector.tensor_scalar_add(
                            dest[:, p, sc * 512:(sc + 1) * 512], ps_t[:],
                            bqk_sb[:, 2 * qk + p: 2 * qk + p + 1])
                for stl in range(4):
                    st = sc * 4 + stl
                    pv_t = ps.tile([128, 256], f32, tag="a")
                    for dc in range(8):
                        nc.tensor.matmul(
                            pv_t[:], lhsT=xts[dc][:, stl * 128:(stl + 1) * 128],
                            rhs=Wv_sb[:, dc, :], start=(dc == 0), stop=False)
                    nc.tensor.matmul(pv_t[:], lhsT=ones[:], rhs=bv_sb[:],
                                     start=False, stop=True)
                    nc.vector.tensor_copy(
                        v4[:, :, st, 0:64],
                        pv_t[:].rearrange("p (h j) -> p h j", h=4))

            # ---- attention per head ----
            for p in range(2):
                for hp in range(2):
                    h = p * 2 + hp
                    ctx_ps = ps.tile([65, S], f32, tag="ctx", bufs=1)
                    for half in range(2):
                        for kt in range(16):
                            sc_t = ps.tile([128, 1024], f32, tag="a")
                            for qc in range(2):
                                q0 = half * 1024 + qc * 512
                                nc.tensor.matmul(
                                    sc_t[:, qc * 512:(qc + 1) * 512],
                                    lhsT=Kt_sb[hp * 64:(hp + 1) * 64, p,
                                               kt * 128:(kt + 1) * 128],
                                    rhs=Qt_sb[hp * 64:(hp + 1) * 64, p, q0:q0 + 512],
                                    start=True, stop=True)
                            et = expp.tile([128, 1024], bf16, tag="et")
                            nc.scalar.activation(
                                et[:], sc_t[:], Exp,
                                bias=maskT_sb[:, kt * 4 + h: kt * 4 + h + 1],
                                scale=1.0)
                            for qc in range(2):
                                q0 = half * 1024 + qc * 512
                                nc.tensor.matmul(
                                    ctx_ps[:, q0:q0 + 512], lhsT=v4[:, h, kt, :],
                                    rhs=et[:, qc * 512:(qc + 1) * 512],
                                    start=(kt == 0), stop=(kt == 15))
                    # head epilogue: move ctx out of PSUM, normalize
                    ctxU = ctxu.tile([65, S], f32, tag="cu")
                    nc.vector.tensor_copy(ctxU[:], ctx_ps[:])
                    s128 = rscr.tile([128, 16], f32, tag="sm")
                    nc.sync.dma_start(s128[:], ctxU[64:65, :])
                    r128 = rscr.tile([128, 16], f32r, tag="r128")
                    with nc.allow_low_precision(reason="f32r rounding for matmul rhs"):
                        nc.vector.reciprocal(r128[:], s128[:])
                    rs_t = rscr.tile([1, S], f32r, tag="rs")
                    nc.sync.dma_start(rs_t[:], r128[:])
                    if hp == 1:
                        ctxN = ctxu.tile([64, S], f32r, tag="cn", bufs=1)
                    for qc in range(4):
                        pb_t = ps.tile([128, 512], f32, tag="a")
                        nc.tensor.matmul(pb_t[:], lhsT=ones[:],
                                         rhs=rs_t[0:1, qc * 512:(qc + 1) * 512],
                                         start=True, stop=True)
                        if hp == 0:
                            tt_out = ctxT_norm[0:64, p, qc * 512:(qc + 1) * 512]
                        else:
                            tt_out = ctxN[0:64, qc * 512:(qc + 1) * 512]
                        nc.vector.tensor_mul(
                            tt_out,
                            ctxU[0:64, qc * 512:(qc + 1) * 512],
                            pb_t[0:64, :])
                    if hp == 1:
                        nc.sync.dma_start(ctxT_norm[64:128, p, :], ctxN[0:64, :])

            # ---- output projection ----
            for qt in range(16):
                po = ps.tile([128, 1024], f32, tag="a")
                for p in range(2):
                    for ec in range(2):
                        nc.tensor.matmul(
                            po[:, ec * 512:(ec + 1) * 512],
                            lhsT=ctxT_norm[:, p, qt * 128:(qt + 1) * 128],
                            rhs=Wo_sb[:, p, ec * 512:(ec + 1) * 512],
                            start=(p == 0), stop=(p == 1))
                ob = outsb.tile([128, 1024], f32, tag="ob")
                nc.vector.tensor_copy(ob[:], po[:])
                nc.sync.dma_start(out_d[qt * 128:(qt + 1) * 128, :], ob[:])

    nc.compile()
    return nc


def get_program():
    if "nc" not in _cache:
        _cache["nc"] = _build_program()
    return _cache["nc"]


def make_in_maps(query, mask, W_qkv, b_qkv, W_out, b_out):
    query = np.asarray(query, dtype=np.float32)
    mask = np.asarray(mask)
    W_qkv = np.asarray(W_qkv, dtype=np.float32)
    b_qkv = np.asarray(b_qkv, dtype=np.float32)
    W_out = np.asarray(W_out, dtype=np.float32)

    W3 = W_qkv.reshape(DIM, N_HEADS, DIM_PER_HEAD, 3)
    b3 = b_qkv.reshape(N_HEADS, DIM_PER_HEAD, 3)
    maskadd = np.where(mask[:, 0, :], np.float32(-30000.0), np.float32(0.0))

    in_maps = []
    for c in range(N_CORES):
        b = c // 4
        h0 = (c % 4) * HEADS_PER_CORE
        hs = slice(h0, h0 + HEADS_PER_CORE)
        bf = np.float16
        Wq_c = np.ascontiguousarray(
            W3[:, hs, :, 0].reshape(DIM, 256) / SCALE).astype(bf)
        Wk_c = np.ascontiguousarray(W3[:, hs, :, 1].reshape(DIM, 256)).astype(bf)
        Wv_c = np.ascontiguousarray(W3[:, hs, :, 2].reshape(DIM, 256)).astype(bf)
        bq_c = (b3[hs, :, 0].reshape(256) / SCALE).astype(np.float32)
        bk_c = b3[hs, :, 1].reshape(256).astype(np.float32)
        bv_c = b3[hs, :, 2].reshape(1, 256).astype(np.float32)
        bqk_c = np.ascontiguousarray(
            np.stack([bq_c[:128], bq_c[128:], bk_c[:128], bk_c[128:]], axis=1))
        Wo_c = np.ascontiguousarray(W_out[h0 * 64:(h0 + 4) * 64, :])
        xT_c = np.ascontiguousarray(query[b].T).astype(bf)
        ma_c = maskadd[b * N_HEADS + h0: b * N_HEADS + h0 + 4]  # [4, 2048]
        maskT_c = np.ascontiguousarray(
            ma_c.reshape(4, 16, 128).transpose(2, 1, 0).reshape(128, 64)
        ).astype(np.float32)
        in_maps.append({
            "xT": xT_c, "Wq": Wq_c, "Wk": Wk_c, "Wv": Wv_c, "Wo": Wo_c,
            "bqk": bqk_c, "bv": bv_c, "maskT": maskT_c,
            "ones_d": np.ones((1, 128), dtype=np.float32),
            "ones_bf": np.ones((128, 64), dtype=bf),
        })
    return in_maps


def gather_outputs(results, b_out):
    b_out = np.asarray(b_out, dtype=np.float32)
    out = np.zeros((B, S, DIM), dtype=np.float32)
    for c in range(N_CORES):
        out[c // 4] += results[c]["out"]
    out += b_out[None, None, :]
    return out


def kernel(query, mask, W_qkv, b_qkv, W_out, b_out):
    from concourse.bass_utils import run_bass_kernel_spmd

    nc = get_program()
    in_maps = make_in_maps(query, mask, W_qkv, b_qkv, W_out, b_out)
    res = run_bass_kernel_spmd(nc, in_maps, list(range(N_CORES)))
    return gather_outputs(res.results, b_out)


# revision 16
# speedup vs baseline: 1.2082x; 1.0192x over previous
"""Trainium2 Bass kernel for nn_MultiHeadAttention (B=2, S=2048, D=1024, H=16).

Sharding: batch*heads across 8 cores -> each core handles one batch element's
4 heads (core c: b = c//4, heads h0 = (c%4)*4 .. h0+4).

Per-core device program (all matmuls float32r, full-rate on PE):
  1. QKV projection from pre-transposed activations xT [1024, 2048]:
       Qt/Kt produced transposed ([q_dim, s], 2 heads stacked per 128
       partitions), V produced natural ([s, v_dim], 4 heads side by side)
       with an extra ones column (row-sum trick) appended per head.
  2. Attention per head in "scoresT" layout [key, query]: PE computes
       scoresT tiles, ScalarE applies additive mask + exp in one op
       (bias is per-partition = per-key), PE contracts exp-scores with
       Vaug -> unnormalized ctxT [65, q] where row 64 is the softmax sum.
  3. Normalize: reciprocal of sums, broadcast across partitions via a K=1
       ones-outer-product matmul, multiply on VectorE.
  4. Output projection: ctxT pairs (128 head-dims) x W_out rows -> partial
       output [2048, 1024], DMA'd straight from PSUM to DRAM.
Host sums the 4 partial outputs per batch element and adds b_out.
"""

import math

import numpy as np

N_HEADS = 16
DIM = 1024
DIM_PER_HEAD = 64
B = 2
S = 2048
SCALE = math.sqrt(DIM_PER_HEAD)
N_CORES = 8
HEADS_PER_CORE = 4

_cache = {}


def _build_program():
    import concourse.bass as bass
    import concourse.tile as tile
    from concourse import bacc, mybir

    f32 = mybir.dt.float32
    f32r = mybir.dt.float32r
    bf16 = mybir.dt.float16
    Exp = mybir.ActivationFunctionType.Exp

    nc = bacc.Bacc("TRN2", target_bir_lowering=False, debug=False,
                   num_devices=N_CORES)

    xT = nc.dram_tensor("xT", [DIM, S], bf16, kind="ExternalInput").ap()
    Wq = nc.dram_tensor("Wq", [DIM, 256], bf16, kind="ExternalInput").ap()
    Wk = nc.dram_tensor("Wk", [DIM, 256], bf16, kind="ExternalInput").ap()
    Wv = nc.dram_tensor("Wv", [DIM, 256], bf16, kind="ExternalInput").ap()
    Wo = nc.dram_tensor("Wo", [256, DIM], bf16, kind="ExternalInput").ap()
    bqk = nc.dram_tensor("bqk", [128, 4], f32, kind="ExternalInput").ap()
    bv = nc.dram_tensor("bv", [1, 256], bf16, kind="ExternalInput").ap()
    maskT = nc.dram_tensor("maskT", [128, 64], f32, kind="ExternalInput").ap()
    ones_d = nc.dram_tensor("ones_d", [1, 512], bf16, kind="ExternalInput").ap()
    ones_bf = nc.dram_tensor("ones_bf", [128, 64], bf16, kind="ExternalInput").ap()
    out_d = nc.dram_tensor("out", [S, DIM], f32, kind="ExternalOutput").ap()

    with tile.TileContext(nc) as tc:
        with tc.tile_pool(name="const", bufs=1) as cpool, \
             tc.tile_pool(name="wpool", bufs=1) as wpool, \
             tc.tile_pool(name="qkv", bufs=1) as qkvp, \
             tc.tile_pool(name="xsub", bufs=10) as xsub, \
             tc.tile_pool(name="expp", bufs=3) as expp, \
             tc.tile_pool(name="ctxu", bufs=2) as ctxu, \
             tc.tile_pool(name="outsb", bufs=2) as outsb, \
             tc.tile_pool(name="rscr", bufs=1) as rscr, \
             tc.tile_pool(name="ps", bufs=2, space="PSUM") as ps:

            ones = cpool.tile([1, 512], bf16)
            nc.sync.dma_start(ones[:], ones_d[:])
            bqk_sb = cpool.tile([128, 4], f32)
            nc.sync.dma_start(bqk_sb[:], bqk[:])
            bv_sb = cpool.tile([1, 256], bf16)
            nc.sync.dma_start(bv_sb[:], bv[:])
            maskT_sb = cpool.tile([128, 64], f32)
            nc.sync.dma_start(maskT_sb[:], maskT[:])

            Wq_sb = wpool.tile([128, 8, 256], bf16)
            nc.sync.dma_start(Wq_sb[:], Wq.rearrange("(c p) j -> p c j", p=128))
            Wk_sb = wpool.tile([128, 8, 256], bf16)
            nc.sync.dma_start(Wk_sb[:], Wk.rearrange("(c p) j -> p c j", p=128))
            Wv_sb = wpool.tile([128, 8, 256], bf16)
            nc.sync.dma_start(Wv_sb[:], Wv.rearrange("(c p) j -> p c j", p=128))
            Wo_sb = wpool.tile([128, 2, 1024], bf16)
            nc.sync.dma_start(Wo_sb[:], Wo.rearrange("(c p) e -> p c e", p=128))

            Qt_sb = qkvp.tile([128, 2, S], bf16)
            Kt_sb = qkvp.tile([128, 2, S], bf16)
            Vaug = qkvp.tile([128, 4 * 16 * 65], bf16)
            v4 = Vaug.rearrange("p (h k j) -> p h k j", h=4, k=16)
            nc.sync.dma_start(v4[:, :, :, 64],
                              ones_bf[:, :].rearrange("p (h k) -> p h k", h=4))
            ctxT_norm = qkvp.tile([128, 2, S], bf16)

            # ---- PE warm-up during the input DMA ramp (keeps HAM at 2.4GHz) ----
            warm_ps = ps.tile([128, 512], f32, tag="a", name="warm_ps")
            for _wi in range(96):
                nc.tensor.matmul(warm_ps[:], lhsT=ones[0:1, 0:128],
                                 rhs=ones[0:1, :], start=True, stop=True)

            # ---- QKV projection ----
            for sc in range(4):
                xts = []
                for dc in range(8):
                    xt_t = xsub.tile([128, 512], bf16, tag="x")
                    nc.sync.dma_start(
                        xt_t[:], xT[dc * 128:(dc + 1) * 128, sc * 512:(sc + 1) * 512])
                    xts.append(xt_t)
                for qk in range(2):
                    W = (Wq_sb, Wk_sb)[qk]
                    dest = (Qt_sb, Kt_sb)[qk]
                    for p in range(2):
                        ps_t = ps.tile([128, 512], f32, tag="a")
                        for dc in range(8):
                            nc.tensor.matmul(
                                ps_t[:], lhsT=W[:, dc, p * 128:(p + 1) * 128],
                                rhs=xts[dc][:], start=(dc == 0), stop=(dc == 7))
                        nc.vector.tensor_scalar_add(
                            dest[:, p, sc * 512:(sc + 1) * 512], ps_t[:],
                            bqk_sb[:, 2 * qk + p: 2 * qk + p + 1])
                for stl in range(4):
                    st = sc * 4 + stl
                    pv_t = ps.tile([128, 256], f32, tag="a")
                    for dc in range(8):
                        nc.tensor.matmul(
                            pv_t[:], lhsT=xts[dc][:, stl * 128:(stl + 1) * 128],
                            rhs=Wv_sb[:, dc, :], start=(dc == 0), stop=False)
                    nc.tensor.matmul(pv_t[:], lhsT=ones[0:1, 0:128], rhs=bv_sb[:],
                                     start=False, stop=True)
                    nc.vector.tensor_copy(
                        v4[:, :, st, 0:64],
                        pv_t[:].rearrange("p (h j) -> p h j", h=4))

            # ---- attention per head ----
            # The normalize work (broadcast matmuls + TT muls) for head h is
            # deferred into head h+1's main loop so the PE never idles on the
            # reciprocal chain (keeps the HAM clock warm).
            def emit_normalize(state):
                p_, hp_, ctxU_, rs_ = state
                ctxN_ = None
                if hp_ == 1:
                    ctxN_ = ctxu.tile([64, S], bf16, tag="cn", bufs=1,
                                      name=f"ctxN_{p_}")
                for qc in range(4):
                    pb_t = ps.tile([128, 512], f32, tag="a", name=f"pb_{p_}_{hp_}_{qc}")
                    nc.tensor.matmul(pb_t[:], lhsT=ones[0:1, 0:128],
                                     rhs=rs_[0:1, qc * 512:(qc + 1) * 512],
                                     start=True, stop=True)
                    if hp_ == 0:
                        tt_out = ctxT_norm[0:64, p_, qc * 512:(qc + 1) * 512]
                    else:
                        tt_out = ctxN_[0:64, qc * 512:(qc + 1) * 512]
                    nc.vector.tensor_mul(
                        tt_out,
                        ctxU_[0:64, qc * 512:(qc + 1) * 512],
                        pb_t[0:64, :])
                if hp_ == 1:
                    nc.sync.dma_start(ctxT_norm[64:128, p_, :], ctxN_[0:64, :])

            pending = None
            for p in range(2):
                for hp in range(2):
                    h = p * 2 + hp
                    ctx_ps = ps.tile([65, S], f32, tag="ctx", bufs=1)
                    for half in range(2):
                        for kt in range(16):
                            sc_t = ps.tile([128, 1024], f32, tag="a")
                            for qc in range(2):
                                q0 = half * 1024 + qc * 512
                                nc.tensor.matmul(
                                    sc_t[:, qc * 512:(qc + 1) * 512],
                                    lhsT=Kt_sb[hp * 64:(hp + 1) * 64, p,
                                               kt * 128:(kt + 1) * 128],
                                    rhs=Qt_sb[hp * 64:(hp + 1) * 64, p, q0:q0 + 512],
                                    start=True, stop=True)
                            et = expp.tile([128, 1024], bf16, tag="et")
                            nc.scalar.activation(
                                et[:], sc_t[:], Exp,
                                bias=maskT_sb[:, kt * 4 + h: kt * 4 + h + 1],
                                scale=1.0)
                            for qc in range(2):
                                q0 = half * 1024 + qc * 512
                                nc.tensor.matmul(
                                    ctx_ps[:, q0:q0 + 512], lhsT=v4[:, h, kt, :],
                                    rhs=et[:, qc * 512:(qc + 1) * 512],
                                    start=(kt == 0), stop=(kt == 15))
                            if (half, kt) == (0, 3) and pending is not None:
                                emit_normalize(pending)
                                pending = None
                    # head drain: move ctx out of PSUM, compute 1/sums
                    ctxU = ctxu.tile([65, S], f32, tag="cu")
                    nc.vector.tensor_copy(ctxU[:], ctx_ps[:])
                    s128 = rscr.tile([128, 16], f32, tag="sm")
                    nc.sync.dma_start(s128[:], ctxU[64:65, :])
                    r128 = rscr.tile([128, 16], bf16, tag="r128")
                    with nc.allow_low_precision(reason="fp16 rounding for matmul rhs"):
                        nc.vector.reciprocal(r128[:], s128[:])
                    rs_t = rscr.tile([1, S], bf16, tag="rs")
                    nc.sync.dma_start(rs_t[:], r128[:])
                    pending = (p, hp, ctxU, rs_t)
            emit_normalize(pending)

            # ---- output projection ----
            for qt in range(16):
                po = ps.tile([128, 1024], f32, tag="a")
                for p in range(2):
                    for ec in range(2):
                        nc.tensor.matmul(
                            po[:, ec * 512:(ec + 1) * 512],
                            lhsT=ctxT_norm[:, p, qt * 128:(qt + 1) * 128],
                            rhs=Wo_sb[:, p, ec * 512:(ec + 1) * 512],
                            start=(p == 0), stop=(p == 1))
                ob = outsb.tile([128, 1024], f32, tag="ob")
                nc.vector.tensor_copy(ob[:], po[:])
                nc.sync.dma_start(out_d[qt * 128:(qt + 1) * 128, :], ob[:])

    nc.compile()
    return nc


def get_program():
    if "nc" not in _cache:
        _cache["nc"] = _build_program()
    return _cache["nc"]


def make_in_maps(query, mask, W_qkv, b_qkv, W_out, b_out):
    query = np.asarray(query, dtype=np.float32)
    mask = np.asarray(mask)
    W_qkv = np.asarray(W_qkv, dtype=np.float32)
    b_qkv = np.asarray(b_qkv, dtype=np.float32)
    W_out = np.asarray(W_out, dtype=np.float32)

    W3 = W_qkv.reshape(DIM, N_HEADS, DIM_PER_HEAD, 3)
    b3 = b_qkv.reshape(N_HEADS, DIM_PER_HEAD, 3)
    maskadd = np.where(mask[:, 0, :], np.float32(-30000.0), np.float32(0.0))

    in_maps = []
    for c in range(N_CORES):
        b = c // 4
        h0 = (c % 4) * HEADS_PER_CORE
        hs = slice(h0, h0 + HEADS_PER_CORE)
        bf = np.float16
        Wq_c = np.ascontiguousarray(
            W3[:, hs, :, 0].reshape(DIM, 256) / SCALE).astype(bf)
        Wk_c = np.ascontiguousarray(W3[:, hs, :, 1].reshape(DIM, 256)).astype(bf)
        Wv_c = np.ascontiguousarray(W3[:, hs, :, 2].reshape(DIM, 256)).astype(bf)
        bq_c = (b3[hs, :, 0].reshape(256) / SCALE).astype(np.float32)
        bk_c = b3[hs, :, 1].reshape(256).astype(np.float32)
        bv_c = b3[hs, :, 2].reshape(1, 256).astype(bf)
        bqk_c = np.ascontiguousarray(
            np.stack([bq_c[:128], bq_c[128:], bk_c[:128], bk_c[128:]], axis=1))
        Wo_c = np.ascontiguousarray(W_out[h0 * 64:(h0 + 4) * 64, :]).astype(bf)
        xT_c = np.ascontiguousarray(query[b].T).astype(bf)
        ma_c = maskadd[b * N_HEADS + h0: b * N_HEADS + h0 + 4]  # [4, 2048]
        maskT_c = np.ascontiguousarray(
            ma_c.reshape(4, 16, 128).transpose(2, 1, 0).reshape(128, 64)
        ).astype(np.float32)
        in_maps.append({
            "xT": xT_c, "Wq": Wq_c, "Wk": Wk_c, "Wv": Wv_c, "Wo": Wo_c,
            "bqk": bqk_c, "bv": bv_c, "maskT": maskT_c,
            "ones_d": np.ones((1, 512), dtype=bf),
            "ones_bf": np.ones((128, 64), dtype=bf),
        })
    return in_maps


def gather_outputs(results, b_out):
    b_out = np.asarray(b_out, dtype=np.float32)
    out = np.zeros((B, S, DIM), dtype=np.float32)
    for c in range(N_CORES):
        out[c // 4] += results[c]["out"]
    out += b_out[None, None, :]
    return out


def kernel(query, mask, W_qkv, b_qkv, W_out, b_out):
    from concourse.bass_utils import run_bass_kernel_spmd

    nc = get_program()
    in_maps = make_in_maps(query, mask, W_qkv, b_qkv, W_out, b_out)
    res = run_bass_kernel_spmd(nc, in_maps, list(range(N_CORES)))
    return gather_outputs(res.results, b_out)


# revision 19
# speedup vs baseline: 1.3544x; 1.1209x over previous
"""Trainium2 Bass kernel for nn_MultiHeadAttention (B=2, S=2048, D=1024, H=16).

Sharding: batch*heads across 8 cores -> each core handles one batch element's
4 heads (core c: b = c//4, heads h0 = (c%4)*4 .. h0+4).

Per-core device program (all matmuls float32r, full-rate on PE):
  1. QKV projection from pre-transposed activations xT [1024, 2048]:
       Qt/Kt produced transposed ([q_dim, s], 2 heads stacked per 128
       partitions), V produced natural ([s, v_dim], 4 heads side by side)
       with an extra ones column (row-sum trick) appended per head.
  2. Attention per head in "scoresT" layout [key, query]: PE computes
       scoresT tiles, ScalarE applies additive mask + exp in one op
       (bias is per-partition = per-key), PE contracts exp-scores with
       Vaug -> unnormalized ctxT [65, q] where row 64 is the softmax sum.
  3. Normalize: reciprocal of sums, broadcast across partitions via a K=1
       ones-outer-product matmul, multiply on VectorE.
  4. Output projection: ctxT pairs (128 head-dims) x W_out rows -> partial
       output [2048, 1024], DMA'd straight from PSUM to DRAM.
Host sums the 4 partial outputs per batch element and adds b_out.
"""

import math

import numpy as np

N_HEADS = 16
DIM = 1024
DIM_PER_HEAD = 64
B = 2
S = 2048
SCALE = math.sqrt(DIM_PER_HEAD)
N_CORES = 8
HEADS_PER_CORE = 4

_cache = {}


def _build_program():
    import concourse.bass as bass
    import concourse.tile as tile
    from concourse import bacc, mybir

    f32 = mybir.dt.float32
    f32r = mybir.dt.float32r
    bf16 = mybir.dt.float16
    Exp = mybir.ActivationFunctionType.Exp

    nc = bacc.Bacc("TRN2", target_bir_lowering=False, debug=False,
                   num_devices=N_CORES)

    xT = nc.dram_tensor("xT", [DIM, S], bf16, kind="ExternalInput").ap()
    Wq = nc.dram_tensor("Wq", [DIM, 256], bf16, kind="ExternalInput").ap()
    Wk = nc.dram_tensor("Wk", [DIM, 256], bf16, kind="ExternalInput").ap()
    Wv = nc.dram_tensor("Wv", [DIM, 256], bf16, kind="ExternalInput").ap()
    Wo = nc.dram_tensor("Wo", [256, DIM], bf16, kind="ExternalInput").ap()
    bqk = nc.dram_tensor("bqk", [128, 4], f32, kind="ExternalInput").ap()
    bv = nc.dram_tensor("bv", [1, 256], bf16, kind="ExternalInput").ap()
    maskT = nc.dram_tensor("maskT", [128, 64], f32, kind="ExternalInput").ap()
    ones_d = nc.dram_tensor("ones_d", [1, 512], bf16, kind="ExternalInput").ap()
    ones_bf = nc.dram_tensor("ones_bf", [128, 64], bf16, kind="ExternalInput").ap()
    out_d = nc.dram_tensor("out", [S, DIM], f32, kind="ExternalOutput").ap()

    with tile.TileContext(nc) as tc:
        with tc.tile_pool(name="const", bufs=1) as cpool, \
             tc.tile_pool(name="wpool", bufs=1) as wpool, \
             tc.tile_pool(name="qkv", bufs=1) as qkvp, \
             tc.tile_pool(name="xsub", bufs=16) as xsub, \
             tc.tile_pool(name="expp", bufs=3) as expp, \
             tc.tile_pool(name="ctxu", bufs=2) as ctxu, \
             tc.tile_pool(name="outsb", bufs=2) as outsb, \
             tc.tile_pool(name="rscr", bufs=1) as rscr, \
             tc.tile_pool(name="ps", bufs=2, space="PSUM") as ps:

            ones = cpool.tile([1, 512], bf16)
            nc.sync.dma_start(ones[:], ones_d[:])
            bqk_sb = cpool.tile([128, 4], f32)
            nc.sync.dma_start(bqk_sb[:], bqk[:])
            bv_sb = cpool.tile([1, 256], bf16)
            nc.sync.dma_start(bv_sb[:], bv[:])
            maskT_sb = cpool.tile([128, 64], f32)
            nc.sync.dma_start(maskT_sb[:], maskT[:])

            Wq_sb = wpool.tile([128, 8, 256], bf16)
            nc.sync.dma_start(Wq_sb[:], Wq.rearrange("(c p) j -> p c j", p=128))
            Wk_sb = wpool.tile([128, 8, 256], bf16)
            nc.sync.dma_start(Wk_sb[:], Wk.rearrange("(c p) j -> p c j", p=128))
            Wv_sb = wpool.tile([128, 8, 256], bf16)
            nc.sync.dma_start(Wv_sb[:], Wv.rearrange("(c p) j -> p c j", p=128))
            Wo_sb = wpool.tile([128, 2, 1024], bf16)
            nc.sync.dma_start(Wo_sb[:], Wo.rearrange("(c p) e -> p c e", p=128))

            Qt_sb = qkvp.tile([128, 2, S], bf16)
            Kt_sb = qkvp.tile([128, 2, S], bf16)
            Vaug = qkvp.tile([128, 4 * 16 * 65], bf16)
            v4 = Vaug.rearrange("p (h k j) -> p h k j", h=4, k=16)
            nc.sync.dma_start(v4[:, :, :, 64],
                              ones_bf[:, :].rearrange("p (h k) -> p h k", h=4))
            ctxT_norm = qkvp.tile([128, 2, S], bf16)

            # ---- QKV projection ----
            for sc in range(4):
                xts = []
                for dc in range(8):
                    xt_t = xsub.tile([128, 512], bf16, tag="x")
                    nc.sync.dma_start(
                        xt_t[:], xT[dc * 128:(dc + 1) * 128, sc * 512:(sc + 1) * 512])
                    xts.append(xt_t)
                for qk in range(2):
                    W = (Wq_sb, Wk_sb)[qk]
                    dest = (Qt_sb, Kt_sb)[qk]
                    for p in range(2):
                        ps_t = ps.tile([128, 512], f32, tag="a")
                        for dc in range(8):
                            nc.tensor.matmul(
                                ps_t[:], lhsT=W[:, dc, p * 128:(p + 1) * 128],
                                rhs=xts[dc][:], start=(dc == 0), stop=(dc == 7))
                        nc.vector.tensor_scalar_add(
                            dest[:, p, sc * 512:(sc + 1) * 512], ps_t[:],
                            bqk_sb[:, 2 * qk + p: 2 * qk + p + 1])
                for stl in range(4):
                    st = sc * 4 + stl
                    pv_t = ps.tile([128, 256], f32, tag="a")
                    for dc in range(8):
                        nc.tensor.matmul(
                            pv_t[:], lhsT=xts[dc][:, stl * 128:(stl + 1) * 128],
                            rhs=Wv_sb[:, dc, :], start=(dc == 0), stop=False)
                    nc.tensor.matmul(pv_t[:], lhsT=ones[0:1, 0:128], rhs=bv_sb[:],
                                     start=False, stop=True)
                    nc.vector.tensor_copy(
                        v4[:, :, st, 0:64],
                        pv_t[:].rearrange("p (h j) -> p h j", h=4))

            # ---- attention per head ----
            # Software-pipelined: scores for step i+1 are issued on the PE
            # while ScalarE runs exp for step i, so the PE never waits on the
            # ACT->PE dependency and consecutive matmuls hide their drain
            # latency. The normalize work for head h (broadcast matmuls + TT
            # muls) is spread one chunk at a time into head h+1's main loop.
            def make_norm_steps(p_, hp_, ctxU_, rs_):
                box = {}

                def step(qc):
                    if hp_ == 1 and "t" not in box:
                        box["t"] = ctxu.tile([64, S], bf16, tag="cn", bufs=1,
                                             name=f"ctxN_{p_}")
                    pb_t = ps.tile([128, 512], f32, tag="a",
                                   name=f"pb_{p_}_{hp_}_{qc}")
                    nc.tensor.matmul(pb_t[:], lhsT=ones[0:1, 0:128],
                                     rhs=rs_[0:1, qc * 512:(qc + 1) * 512],
                                     start=True, stop=True)
                    if hp_ == 0:
                        tt_out = ctxT_norm[0:64, p_, qc * 512:(qc + 1) * 512]
                    else:
                        tt_out = box["t"][0:64, qc * 512:(qc + 1) * 512]
                    nc.vector.tensor_mul(
                        tt_out,
                        ctxU_[0:64, qc * 512:(qc + 1) * 512],
                        pb_t[0:64, :])
                    if hp_ == 1 and qc == 3:
                        nc.sync.dma_start(ctxT_norm[64:128, p_, :],
                                          box["t"][0:64, :])

                return [lambda qc=qc: step(qc) for qc in range(4)]

            SEQ = [(half, kt) for half in range(2) for kt in range(16)]

            norm_steps = []
            for p in range(2):
                for hp in range(2):
                    h = p * 2 + hp
                    ctx_ps = ps.tile([65, S], f32, tag="ctx", bufs=1)

                    def emit_scores(half, kt):
                        sc_t = ps.tile([128, 1024], f32, tag="a",
                                       name=f"sc_{h}_{half}_{kt}")
                        for qc in range(2):
                            q0 = half * 1024 + qc * 512
                            nc.tensor.matmul(
                                sc_t[:, qc * 512:(qc + 1) * 512],
                                lhsT=Kt_sb[hp * 64:(hp + 1) * 64, p,
                                           kt * 128:(kt + 1) * 128],
                                rhs=Qt_sb[hp * 64:(hp + 1) * 64, p, q0:q0 + 512],
                                start=True, stop=True)
                        return sc_t

                    sc_cur = emit_scores(*SEQ[0])
                    for i, (half, kt) in enumerate(SEQ):
                        et = expp.tile([128, 1024], bf16, tag="et")
                        nc.scalar.activation(
                            et[:], sc_cur[:], Exp,
                            bias=maskT_sb[:, kt * 4 + h: kt * 4 + h + 1],
                            scale=1.0)
                        if i + 1 < len(SEQ):
                            sc_cur = emit_scores(*SEQ[i + 1])
                        for qc in range(2):
                            q0 = half * 1024 + qc * 512
                            nc.tensor.matmul(
                                ctx_ps[:, q0:q0 + 512], lhsT=v4[:, h, kt, :],
                                rhs=et[:, qc * 512:(qc + 1) * 512],
                                start=(kt == 0), stop=(kt == 15))
                        if norm_steps and 3 <= i <= 6:
                            norm_steps.pop(0)()
                    # head drain: move ctx out of PSUM, compute 1/sums
                    ctxU = ctxu.tile([65, S], f32, tag="cu")
                    nc.vector.tensor_copy(ctxU[:], ctx_ps[:])
                    s128 = rscr.tile([128, 16], f32, tag="sm")
                    nc.sync.dma_start(s128[:], ctxU[64:65, :])
                    r128 = rscr.tile([128, 16], bf16, tag="r128")
                    with nc.allow_low_precision(reason="fp16 rounding for matmul rhs"):
                        nc.vector.reciprocal(r128[:], s128[:])
                    rs_t = rscr.tile([1, S], bf16, tag="rs")
                    nc.sync.dma_start(rs_t[:], r128[:])
                    norm_steps = make_norm_steps(p, hp, ctxU, rs_t)
            for st in norm_steps:
                st()

            # ---- output projection ----
            for qt in range(16):
                po = ps.tile([128, 1024], f32, tag="a")
                for p in range(2):
                    for ec in range(2):
                        nc.tensor.matmul(
                            po[:, ec * 512:(ec + 1) * 512],
                            lhsT=ctxT_norm[:, p, qt * 128:(qt + 1) * 128],
                            rhs=Wo_sb[:, p, ec * 512:(ec + 1) * 512],
                            start=(p == 0), stop=(p == 1))
                ob = outsb.tile([128, 1024], f32, tag="ob")
                nc.vector.tensor_copy(ob[:], po[:])
                nc.sync.dma_start(out_d[qt * 128:(qt + 1) * 128, :], ob[:])

    nc.compile()
    return nc


def get_program():
    if "nc" not in _cache:
        _cache["nc"] = _build_program()
    return _cache["nc"]


def make_in_maps(query, mask, W_qkv, b_qkv, W_out, b_out):
    query = np.asarray(query, dtype=np.float32)
    mask = np.asarray(mask)
    W_qkv = np.asarray(W_qkv, dtype=np.float32)
    b_qkv = np.asarray(b_qkv, dtype=np.float32)
    W_out = np.asarray(W_out, dtype=np.float32)

    W3 = W_qkv.reshape(DIM, N_HEADS, DIM_PER_HEAD, 3)
    b3 = b_qkv.reshape(N_HEADS, DIM_PER_HEAD, 3)
    maskadd = np.where(mask[:, 0, :], np.float32(-30000.0), np.float32(0.0))

    in_maps = []
    for c in range(N_CORES):
        b = c // 4
        h0 = (c % 4) * HEADS_PER_CORE
        hs = slice(h0, h0 + HEADS_PER_CORE)
        bf = np.float16
        Wq_c = np.ascontiguousarray(
            W3[:, hs, :, 0].reshape(DIM, 256) / SCALE).astype(bf)
        Wk_c = np.ascontiguousarray(W3[:, hs, :, 1].reshape(DIM, 256)).astype(bf)
        Wv_c = np.ascontiguousarray(W3[:, hs, :, 2].reshape(DIM, 256)).astype(bf)
        bq_c = (b3[hs, :, 0].reshape(256) / SCALE).astype(np.float32)
        bk_c = b3[hs, :, 1].reshape(256).astype(np.float32)
        bv_c = b3[hs, :, 2].reshape(1, 256).astype(bf)
        bqk_c = np.ascontiguousarray(
            np.stack([bq_c[:128], bq_c[128:], bk_c[:128], bk_c[128:]], axis=1))
        Wo_c = np.ascontiguousarray(W_out[h0 * 64:(h0 + 4) * 64, :]).astype(bf)
        xT_c = np.ascontiguousarray(query[b].T).astype(bf)
        ma_c = maskadd[b * N_HEADS + h0: b * N_HEADS + h0 + 4]  # [4, 2048]
        maskT_c = np.ascontiguousarray(
            ma_c.reshape(4, 16, 128).transpose(2, 1, 0).reshape(128, 64)
        ).astype(np.float32)
        in_maps.append({
            "xT": xT_c, "Wq": Wq_c, "Wk": Wk_c, "Wv": Wv_c, "Wo": Wo_c,
            "bqk": bqk_c, "bv": bv_c, "maskT": maskT_c,
            "ones_d": np.ones((1, 512), dtype=bf),
            "ones_bf": np.ones((128, 64), dtype=bf),
        })
    return in_maps


def gather_outputs(results, b_out):
    b_out = np.asarray(b_out, dtype=np.float32)
    out = np.zeros((B, S, DIM), dtype=np.float32)
    for c in range(N_CORES):
        out[c // 4] += results[c]["out"]
    out += b_out[None, None, :]
    return out


def kernel(query, mask, W_qkv, b_qkv, W_out, b_out):
    from concourse.bass_utils import run_bass_kernel_spmd

    nc = get_program()
    in_maps = make_in_maps(query, mask, W_qkv, b_qkv, W_out, b_out)
    res = run_bass_kernel_spmd(nc, in_maps, list(range(N_CORES)))
    return gather_outputs(res.results, b_out)


# revision 20
# speedup vs baseline: 1.8509x; 1.3666x over previous
"""Trainium2 Bass kernel for nn_MultiHeadAttention (B=2, S=2048, D=1024, H=16).

Sharding: batch*heads across 8 cores -> each core handles one batch element's
4 heads (core c: b = c//4, heads h0 = (c%4)*4 .. h0+4).

Per-core device program (all matmuls float32r, full-rate on PE):
  1. QKV projection from pre-transposed activations xT [1024, 2048]:
       Qt/Kt produced transposed ([q_dim, s], 2 heads stacked per 128
       partitions), V produced natural ([s, v_dim], 4 heads side by side)
       with an extra ones column (row-sum trick) appended per head.
  2. Attention per head in "scoresT" layout [key, query]: PE computes
       scoresT tiles, ScalarE applies additive mask + exp in one op
       (bias is per-partition = per-key), PE contracts exp-scores with
       Vaug -> unnormalized ctxT [65, q] where row 64 is the softmax sum.
  3. Normalize: reciprocal of sums, broadcast across partitions via a K=1
       ones-outer-product matmul, multiply on VectorE.
  4. Output projection: ctxT pairs (128 head-dims) x W_out rows -> partial
       output [2048, 1024], DMA'd straight from PSUM to DRAM.
Host sums the 4 partial outputs per batch element and adds b_out.
"""

import math

import numpy as np

N_HEADS = 16
DIM = 1024
DIM_PER_HEAD = 64
B = 2
S = 2048
SCALE = math.sqrt(DIM_PER_HEAD)
N_CORES = 8
HEADS_PER_CORE = 4

_cache = {}


def _build_program():
    import concourse.bass as bass
    import concourse.tile as tile
    from concourse import bacc, mybir

    f32 = mybir.dt.float32
    f32r = mybir.dt.float32r
    bf16 = mybir.dt.float16
    Exp = mybir.ActivationFunctionType.Exp

    nc = bacc.Bacc("TRN2", target_bir_lowering=False, debug=False,
                   num_devices=N_CORES)

    xT = nc.dram_tensor("xT", [DIM, S], bf16, kind="ExternalInput").ap()
    Wq = nc.dram_tensor("Wq", [DIM, 256], bf16, kind="ExternalInput").ap()
    Wk = nc.dram_tensor("Wk", [DIM, 256], bf16, kind="ExternalInput").ap()
    Wv = nc.dram_tensor("Wv", [DIM, 256], bf16, kind="ExternalInput").ap()
    Wo = nc.dram_tensor("Wo", [256, DIM], bf16, kind="ExternalInput").ap()
    bqk = nc.dram_tensor("bqk", [128, 4], f32, kind="ExternalInput").ap()
    bv = nc.dram_tensor("bv", [1, 256], bf16, kind="ExternalInput").ap()
    maskT = nc.dram_tensor("maskT", [128, 64], f32, kind="ExternalInput").ap()
    ones_d = nc.dram_tensor("ones_d", [1, 512], bf16, kind="ExternalInput").ap()
    ones_bf = nc.dram_tensor("ones_bf", [128, 64], bf16, kind="ExternalInput").ap()
    out_d = nc.dram_tensor("out", [S, DIM], f32, kind="ExternalOutput").ap()

    with tile.TileContext(nc) as tc:
        with tc.tile_pool(name="const", bufs=1) as cpool, \
             tc.tile_pool(name="wpool", bufs=1) as wpool, \
             tc.tile_pool(name="qkv", bufs=1) as qkvp, \
             tc.tile_pool(name="xsub", bufs=33) as xsub, \
             tc.tile_pool(name="expp", bufs=5) as expp, \
             tc.tile_pool(name="ctxu", bufs=2) as ctxu, \
             tc.tile_pool(name="outsb", bufs=2) as outsb, \
             tc.tile_pool(name="rscr", bufs=2) as rscr, \
             tc.tile_pool(name="ps", bufs=2, space="PSUM") as ps:

            # --- inputs needed first: Qt/Kt weights, bias, activations ---
            bqk_sb = cpool.tile([128, 4], f32)
            nc.sync.dma_start(bqk_sb[:], bqk[:])
            Wq_sb = wpool.tile([128, 8, 256], bf16)
            nc.sync.dma_start(Wq_sb[:], Wq.rearrange("(c p) j -> p c j", p=128))
            Wk_sb = wpool.tile([128, 8, 256], bf16)
            nc.sync.dma_start(Wk_sb[:], Wk.rearrange("(c p) j -> p c j", p=128))

            xts = []
            for sc in range(4):
                for dc in range(8):
                    xt_t = xsub.tile([128, 512], bf16, tag="x",
                                     name=f"x_{sc}_{dc}")
                    nc.sync.dma_start(
                        xt_t[:], xT[dc * 128:(dc + 1) * 128,
                                    sc * 512:(sc + 1) * 512])
                    xts.append(xt_t)

            Qt_sb = qkvp.tile([128, 2, S], bf16)
            Kt_sb = qkvp.tile([128, 2, S], bf16)
            Vaug = qkvp.tile([128, 4 * 16 * 65], bf16)
            v4 = Vaug.rearrange("p (h k j) -> p h k j", h=4, k=16)
            ctxT_norm = qkvp.tile([128, 2, S], bf16)

            # ---- Q/K projection (transposed layout, 2 heads per 128 rows) ----
            for sc in range(4):
                for qk in range(2):
                    W = (Wq_sb, Wk_sb)[qk]
                    dest = (Qt_sb, Kt_sb)[qk]
                    for p in range(2):
                        ps_t = ps.tile([128, 512], f32, tag="a",
                                       name=f"pq_{sc}_{qk}_{p}")
                        for dc in range(8):
                            nc.tensor.matmul(
                                ps_t[:], lhsT=W[:, dc, p * 128:(p + 1) * 128],
                                rhs=xts[sc * 8 + dc][:],
                                start=(dc == 0), stop=(dc == 7))
                        nc.vector.tensor_scalar_add(
                            dest[:, p, sc * 512:(sc + 1) * 512], ps_t[:],
                            bqk_sb[:, 2 * qk + p: 2 * qk + p + 1])

            # --- later inputs (DMAs overlap the Q/K projection above) ---
            Wv_sb = wpool.tile([128, 8, 256], bf16)
            nc.sync.dma_start(Wv_sb[:], Wv.rearrange("(c p) j -> p c j", p=128))
            bv_sb = cpool.tile([1, 256], bf16)
            nc.sync.dma_start(bv_sb[:], bv[:])
            ones = cpool.tile([1, 512], bf16)
            nc.sync.dma_start(ones[:], ones_d[:])
            nc.sync.dma_start(v4[:, :, :, 64],
                              ones_bf[:, :].rearrange("p (h k) -> p h k", h=4))
            maskT_sb = cpool.tile([128, 64], f32)
            nc.sync.dma_start(maskT_sb[:], maskT[:])
            Wo_sb = wpool.tile([128, 2, 1024], bf16)
            nc.sync.dma_start(Wo_sb[:], Wo.rearrange("(c p) e -> p c e", p=128))

            # ---- V projection (natural layout, all 4 heads side by side) ----
            for st in range(16):
                sc, stl = st // 4, st % 4
                pv_t = ps.tile([128, 256], f32, tag="a", name=f"pv_{st}")
                for dc in range(8):
                    nc.tensor.matmul(
                        pv_t[:],
                        lhsT=xts[sc * 8 + dc][:, stl * 128:(stl + 1) * 128],
                        rhs=Wv_sb[:, dc, :], start=(dc == 0), stop=False)
                nc.tensor.matmul(pv_t[:], lhsT=ones[0:1, 0:128], rhs=bv_sb[:],
                                 start=False, stop=True)
                nc.vector.tensor_copy(
                    v4[:, :, st, 0:64],
                    pv_t[:].rearrange("p (h j) -> p h j", h=4))

            # ---- attention, one head PAIR at a time ----
            # scoresT layout [key, query]; the two heads of a pair live at
            # partition bases 0 and 64, so their K=64 score matmuls row-pack
            # and run concurrently on the PE (2x). q is processed in halves of
            # 1024 so both heads' ctx accumulators fit in PSUM. The ctx
            # matmuls for step kt are issued while ScalarE still exps step
            # kt+1 (one-step lag) so the PE never stalls on ACT. The
            # normalize work for a block is deferred into the next block.
            def make_norm_steps(p_, half_, ctxUs_, rss_):
                box = {}

                def step(j):
                    hp_, qc = j // 2, j % 2
                    if hp_ == 1 and "t" not in box:
                        box["t"] = ctxu.tile([64, 1024], bf16, tag="cn",
                                             bufs=2, name=f"ctxN_{p_}_{half_}")
                    pb_t = ps.tile([128, 512], f32, tag="a",
                                   name=f"pb_{p_}_{half_}_{j}")
                    nc.tensor.matmul(pb_t[:], lhsT=ones[0:1, 0:128],
                                     rhs=rss_[hp_][0:1, qc * 512:(qc + 1) * 512],
                                     start=True, stop=True)
                    if hp_ == 0:
                        tt_out = ctxT_norm[0:64, p_,
                                           half_ * 1024 + qc * 512:
                                           half_ * 1024 + (qc + 1) * 512]
                    else:
                        tt_out = box["t"][0:64, qc * 512:(qc + 1) * 512]
                    nc.vector.tensor_mul(
                        tt_out,
                        ctxUs_[hp_][0:64, qc * 512:(qc + 1) * 512],
                        pb_t[0:64, :])
                    if hp_ == 1 and qc == 1:
                        nc.sync.dma_start(
                            ctxT_norm[64:128, p_,
                                      half_ * 1024:(half_ + 1) * 1024],
                            box["t"][0:64, :])

                return [lambda j=j: step(j) for j in range(4)]

            norm_steps = []
            for p in range(2):
                for half in range(2):
                    h0 = p * 2
                    ctx0 = ps.tile([65, 1024], f32, tag="ctx", bufs=2,
                                   name=f"ctx0_{p}_{half}")
                    ctx1 = ps.tile([65, 1024], f32, tag="ctx", bufs=2,
                                   name=f"ctx1_{p}_{half}")
                    ctxs = (ctx0, ctx1)

                    def emit_scores(kt):
                        s0 = ps.tile([128, 1024], f32, tag="a",
                                     name=f"s0_{p}_{half}_{kt}")
                        s1 = ps.tile([128, 1024], f32, tag="a",
                                     name=f"s1_{p}_{half}_{kt}")
                        for qc in range(2):
                            q0 = half * 1024 + qc * 512
                            for hp, s_t in ((0, s0), (1, s1)):
                                nc.tensor.matmul(
                                    s_t[:, qc * 512:(qc + 1) * 512],
                                    lhsT=Kt_sb[hp * 64:(hp + 1) * 64, p,
                                               kt * 128:(kt + 1) * 128],
                                    rhs=Qt_sb[hp * 64:(hp + 1) * 64, p,
                                              q0:q0 + 512],
                                    start=True, stop=True)
                        return s0, s1

                    def emit_ctx(kt, ets):
                        for hp in range(2):
                            for qc in range(2):
                                nc.tensor.matmul(
                                    ctxs[hp][:, qc * 512:(qc + 1) * 512],
                                    lhsT=v4[:, h0 + hp, kt, :],
                                    rhs=ets[hp][:, qc * 512:(qc + 1) * 512],
                                    start=(kt == 0), stop=(kt == 15))

                    sc_cur = emit_scores(0)
                    prev_ets = None
                    for kt in range(16):
                        ets = []
                        for hp in range(2):
                            et = expp.tile([128, 1024], bf16, tag="et",
                                           name=f"et_{p}_{half}_{kt}_{hp}")
                            nc.scalar.activation(
                                et[:], sc_cur[hp][:], Exp,
                                bias=maskT_sb[:, kt * 4 + h0 + hp:
                                              kt * 4 + h0 + hp + 1],
                                scale=1.0)
                            ets.append(et)
                        if prev_ets is not None:
                            emit_ctx(kt - 1, prev_ets)
                        if kt < 15:
                            sc_cur = emit_scores(kt + 1)
                        prev_ets = ets
                        if norm_steps and 3 <= kt <= 6:
                            norm_steps.pop(0)()
                    emit_ctx(15, prev_ets)

                    # drain: move ctx out of PSUM, compute 1/sums per head
                    ctxUs, rss = [], []
                    for hp in range(2):
                        ctxU = ctxu.tile([65, 1024], f32, tag="cu", bufs=4,
                                         name=f"ctxU_{p}_{half}_{hp}")
                        nc.vector.tensor_copy(ctxU[:], ctxs[hp][:])
                        s128 = rscr.tile([128, 8], f32, tag="sm",
                                         name=f"s128_{p}_{half}_{hp}")
                        nc.sync.dma_start(s128[:], ctxU[64:65, :])
                        r128 = rscr.tile([128, 8], bf16, tag="r128",
                                         name=f"r128_{p}_{half}_{hp}")
                        with nc.allow_low_precision(
                                reason="fp16 rounding for matmul rhs"):
                            nc.vector.reciprocal(r128[:], s128[:])
                        rs_t = rscr.tile([1, 1024], bf16, tag="rs",
                                         name=f"rs_{p}_{half}_{hp}")
                        nc.sync.dma_start(rs_t[:], r128[:])
                        ctxUs.append(ctxU)
                        rss.append(rs_t)
                    norm_steps = make_norm_steps(p, half, ctxUs, rss)
            for st_fn in norm_steps:
                st_fn()

            # ---- output projection ----
            for qt in range(16):
                po = ps.tile([128, 1024], f32, tag="a", name=f"po_{qt}")
                for p in range(2):
                    for ec in range(2):
                        nc.tensor.matmul(
                            po[:, ec * 512:(ec + 1) * 512],
                            lhsT=ctxT_norm[:, p, qt * 128:(qt + 1) * 128],
                            rhs=Wo_sb[:, p, ec * 512:(ec + 1) * 512],
                            start=(p == 0), stop=(p == 1))
                ob = outsb.tile([128, 1024], f32, tag="ob", name=f"ob_{qt}")
                nc.vector.tensor_copy(ob[:], po[:])
                nc.sync.dma_start(out_d[qt * 128:(qt + 1) * 128, :], ob[:])

    nc.compile()
    return nc


def get_program():
    if "nc" not in _cache:
        _cache["nc"] = _build_program()
    return _cache["nc"]


def make_in_maps(query, mask, W_qkv, b_qkv, W_out, b_out):
    query = np.asarray(query, dtype=np.float32)
    mask = np.asarray(mask)
    W_qkv = np.asarray(W_qkv, dtype=np.float32)
    b_qkv = np.asarray(b_qkv, dtype=np.float32)
    W_out = np.asarray(W_out, dtype=np.float32)

    W3 = W_qkv.reshape(DIM, N_HEADS, DIM_PER_HEAD, 3)
    b3 = b_qkv.reshape(N_HEADS, DIM_PER_HEAD, 3)
    maskadd = np.where(mask[:, 0, :], np.float32(-30000.0), np.float32(0.0))

    in_maps = []
    for c in range(N_CORES):
        b = c // 4
        h0 = (c % 4) * HEADS_PER_CORE
        hs = slice(h0, h0 + HEADS_PER_CORE)
        bf = np.float16
        Wq_c = np.ascontiguousarray(
            W3[:, hs, :, 0].reshape(DIM, 256) / SCALE).astype(bf)
        Wk_c = np.ascontiguousarray(W3[:, hs, :, 1].reshape(DIM, 256)).astype(bf)
        Wv_c = np.ascontiguousarray(W3[:, hs, :, 2].reshape(DIM, 256)).astype(bf)
        bq_c = (b3[hs, :, 0].reshape(256) / SCALE).astype(np.float32)
        bk_c = b3[hs, :, 1].reshape(256).astype(np.float32)
        bv_c = b3[hs, :, 2].reshape(1, 256).astype(bf)
        bqk_c = np.ascontiguousarray(
            np.stack([bq_c[:128], bq_c[128:], bk_c[:128], bk_c[128:]], axis=1))
        Wo_c = np.ascontiguousarray(W_out[h0 * 64:(h0 + 4) * 64, :]).astype(bf)
        xT_c = np.ascontiguousarray(query[b].T).astype(bf)
        ma_c = maskadd[b * N_HEADS + h0: b * N_HEADS + h0 + 4]  # [4, 2048]
        maskT_c = np.ascontiguousarray(
            ma_c.reshape(4, 16, 128).transpose(2, 1, 0).reshape(128, 64)
        ).astype(np.float32)
        in_maps.append({
            "xT": xT_c, "Wq": Wq_c, "Wk": Wk_c, "Wv": Wv_c, "Wo": Wo_c,
            "bqk": bqk_c, "bv": bv_c, "maskT": maskT_c,
            "ones_d": np.ones((1, 512), dtype=bf),
            "ones_bf": np.ones((128, 64), dtype=bf),
        })
    return in_maps


def gather_outputs(results, b_out):
    b_out = np.asarray(b_out, dtype=np.float32)
    out = np.zeros((B, S, DIM), dtype=np.float32)
    for c in range(N_CORES):
        out[c // 4] += results[c]["out"]
    out += b_out[None, None, :]
    return out


def kernel(query, mask, W_qkv, b_qkv, W_out, b_out):
    from concourse.bass_utils import run_bass_kernel_spmd

    nc = get_program()
    in_maps = make_in_maps(query, mask, W_qkv, b_qkv, W_out, b_out)
    res = run_bass_kernel_spmd(nc, in_maps, list(range(N_CORES)))
    return gather_outputs(res.results, b_out)


# revision 21
# speedup vs baseline: 1.9294x; 1.0424x over previous
"""Trainium2 Bass kernel for nn_MultiHeadAttention (B=2, S=2048, D=1024, H=16).

Sharding: batch*heads across 8 cores -> each core handles one batch element's
4 heads (core c: b = c//4, heads h0 = (c%4)*4 .. h0+4).

Per-core device program (all matmuls float32r, full-rate on PE):
  1. QKV projection from pre-transposed activations xT [1024, 2048]:
       Qt/Kt produced transposed ([q_dim, s], 2 heads stacked per 128
       partitions), V produced natural ([s, v_dim], 4 heads side by side)
       with an extra ones column (row-sum trick) appended per head.
  2. Attention per head in "scoresT" layout [key, query]: PE computes
       scoresT tiles, ScalarE applies additive mask + exp in one op
       (bias is per-partition = per-key), PE contracts exp-scores with
       Vaug -> unnormalized ctxT [65, q] where row 64 is the softmax sum.
  3. Normalize: reciprocal of sums, broadcast across partitions via a K=1
       ones-outer-product matmul, multiply on VectorE.
  4. Output projection: ctxT pairs (128 head-dims) x W_out rows -> partial
       output [2048, 1024], DMA'd straight from PSUM to DRAM.
Host sums the 4 partial outputs per batch element and adds b_out.
"""

import math

import numpy as np

N_HEADS = 16
DIM = 1024
DIM_PER_HEAD = 64
B = 2
S = 2048
SCALE = math.sqrt(DIM_PER_HEAD)
N_CORES = 8
HEADS_PER_CORE = 4

_cache = {}


def _build_program():
    import concourse.bass as bass
    import concourse.tile as tile
    from concourse import bacc, mybir

    f32 = mybir.dt.float32
    f32r = mybir.dt.float32r
    bf16 = mybir.dt.float16
    Exp = mybir.ActivationFunctionType.Exp

    nc = bacc.Bacc("TRN2", target_bir_lowering=False, debug=False,
                   num_devices=N_CORES)

    xT = nc.dram_tensor("xT", [DIM, S], bf16, kind="ExternalInput").ap()
    Wq = nc.dram_tensor("Wq", [DIM, 256], bf16, kind="ExternalInput").ap()
    Wk = nc.dram_tensor("Wk", [DIM, 256], bf16, kind="ExternalInput").ap()
    Wv = nc.dram_tensor("Wv", [DIM, 256], bf16, kind="ExternalInput").ap()
    Wo = nc.dram_tensor("Wo", [256, DIM], bf16, kind="ExternalInput").ap()
    bqk = nc.dram_tensor("bqk", [128, 4], f32, kind="ExternalInput").ap()
    bv = nc.dram_tensor("bv", [1, 256], bf16, kind="ExternalInput").ap()
    maskT = nc.dram_tensor("maskT", [128, 64], f32, kind="ExternalInput").ap()
    ones_d = nc.dram_tensor("ones_d", [1, 512], bf16, kind="ExternalInput").ap()
    ones_bf = nc.dram_tensor("ones_bf", [128, 64], bf16, kind="ExternalInput").ap()
    zeros_d = nc.dram_tensor("zeros_d", [64, 4096], bf16, kind="ExternalInput").ap()
    out_d = nc.dram_tensor("out", [S, DIM], f32, kind="ExternalOutput").ap()

    with tile.TileContext(nc) as tc:
        with tc.tile_pool(name="const", bufs=1) as cpool, \
             tc.tile_pool(name="wpool", bufs=1) as wpool, \
             tc.tile_pool(name="qkv", bufs=1) as qkvp, \
             tc.tile_pool(name="xsub", bufs=33) as xsub, \
             tc.tile_pool(name="expp", bufs=5) as expp, \
             tc.tile_pool(name="ctxu", bufs=2) as ctxu, \
             tc.tile_pool(name="outsb", bufs=2) as outsb, \
             tc.tile_pool(name="rscr", bufs=2) as rscr, \
             tc.tile_pool(name="ps", bufs=2, space="PSUM") as ps:

            # --- inputs needed first: Qt/Kt weights, bias, activations ---
            bqk_sb = cpool.tile([128, 4], f32)
            nc.sync.dma_start(bqk_sb[:], bqk[:])
            Wq_sb = wpool.tile([128, 8, 256], bf16)
            nc.sync.dma_start(Wq_sb[:], Wq.rearrange("(c p) j -> p c j", p=128))
            Wk_sb = wpool.tile([128, 8, 256], bf16)
            nc.sync.dma_start(Wk_sb[:], Wk.rearrange("(c p) j -> p c j", p=128))

            xts = []
            for sc in range(4):
                for dc in range(8):
                    xt_t = xsub.tile([128, 512], bf16, tag="x",
                                     name=f"x_{sc}_{dc}")
                    nc.sync.dma_start(
                        xt_t[:], xT[dc * 128:(dc + 1) * 128,
                                    sc * 512:(sc + 1) * 512])
                    xts.append(xt_t)

            Qt0_sb = qkvp.tile([128, 2, S], bf16)
            Qt1_sb = qkvp.tile([128, 2, S], bf16)
            nc.sync.dma_start(Qt0_sb[64:128, :, :], zeros_d.rearrange("p (c s) -> p c s", c=2))
            nc.sync.dma_start(Qt1_sb[0:64, :, :], zeros_d.rearrange("p (c s) -> p c s", c=2))
            Kt_sb = qkvp.tile([128, 2, S], bf16)
            Vaug = qkvp.tile([128, 4 * 16 * 65], bf16)
            v4 = Vaug.rearrange("p (h k j) -> p h k j", h=4, k=16)
            ctxT_norm = qkvp.tile([128, 2, S], bf16)

            # ---- Q/K projection (transposed layout, 2 heads per 128 rows) ----
            for sc in range(4):
                for qk in range(2):
                    W = (Wq_sb, Wk_sb)[qk]
                    for p in range(2):
                        ps_t = ps.tile([128, 512], f32, tag="a",
                                       name=f"pq_{sc}_{qk}_{p}")
                        for dc in range(8):
                            nc.tensor.matmul(
                                ps_t[:], lhsT=W[:, dc, p * 128:(p + 1) * 128],
                                rhs=xts[sc * 8 + dc][:],
                                start=(dc == 0), stop=(dc == 7))
                        ssl = slice(sc * 512, (sc + 1) * 512)
                        bias = bqk_sb[:, 2 * qk + p: 2 * qk + p + 1]
                        if qk == 1:
                            nc.vector.tensor_scalar_add(
                                Kt_sb[:, p, ssl], ps_t[:], bias)
                        else:
                            nc.vector.tensor_scalar_add(
                                Qt0_sb[0:64, p, ssl], ps_t[0:64, :],
                                bias[0:64, :])
                            nc.vector.tensor_scalar_add(
                                Qt1_sb[64:128, p, ssl], ps_t[64:128, :],
                                bias[64:128, :])

            # --- later inputs (DMAs overlap the Q/K projection above) ---
            Wv_sb = wpool.tile([128, 8, 256], bf16)
            nc.sync.dma_start(Wv_sb[:], Wv.rearrange("(c p) j -> p c j", p=128))
            bv_sb = cpool.tile([1, 256], bf16)
            nc.sync.dma_start(bv_sb[:], bv[:])
            ones = cpool.tile([1, 512], bf16)
            nc.sync.dma_start(ones[:], ones_d[:])
            nc.sync.dma_start(v4[:, :, :, 64],
                              ones_bf[:, :].rearrange("p (h k) -> p h k", h=4))
            maskT_sb = cpool.tile([128, 64], f32)
            nc.sync.dma_start(maskT_sb[:], maskT[:])
            Wo_sb = wpool.tile([128, 2, 1024], bf16)
            nc.sync.dma_start(Wo_sb[:], Wo.rearrange("(c p) e -> p c e", p=128))

            # ---- V projection (natural layout, all 4 heads side by side) ----
            for st in range(16):
                sc, stl = st // 4, st % 4
                pv_t = ps.tile([128, 256], f32, tag="a", name=f"pv_{st}")
                for dc in range(8):
                    nc.tensor.matmul(
                        pv_t[:],
                        lhsT=xts[sc * 8 + dc][:, stl * 128:(stl + 1) * 128],
                        rhs=Wv_sb[:, dc, :], start=(dc == 0), stop=False)
                nc.tensor.matmul(pv_t[:], lhsT=ones[0:1, 0:128], rhs=bv_sb[:],
                                 start=False, stop=True)
                nc.vector.tensor_copy(
                    v4[:, :, st, 0:64],
                    pv_t[:].rearrange("p (h j) -> p h j", h=4))

            # ---- attention, one head PAIR at a time ----
            # scoresT layout [key, query]; the two heads of a pair live at
            # partition bases 0 and 64, so their K=64 score matmuls row-pack
            # and run concurrently on the PE (2x). q is processed in halves of
            # 1024 so both heads' ctx accumulators fit in PSUM. The ctx
            # matmuls for step kt are issued while ScalarE still exps step
            # kt+1 (one-step lag) so the PE never stalls on ACT. The
            # normalize work for a block is deferred into the next block.
            def make_norm_steps(p_, half_, ctxUs_, rss_):
                box = {}

                def step(j):
                    hp_, qc = j // 2, j % 2
                    if hp_ == 1 and "t" not in box:
                        box["t"] = ctxu.tile([64, 1024], bf16, tag="cn",
                                             bufs=2, name=f"ctxN_{p_}_{half_}")
                    pb_t = ps.tile([128, 512], f32, tag="a",
                                   name=f"pb_{p_}_{half_}_{j}")
                    nc.tensor.matmul(pb_t[:], lhsT=ones[0:1, 0:128],
                                     rhs=rss_[hp_][0:1, qc * 512:(qc + 1) * 512],
                                     start=True, stop=True)
                    if hp_ == 0:
                        tt_out = ctxT_norm[0:64, p_,
                                           half_ * 1024 + qc * 512:
                                           half_ * 1024 + (qc + 1) * 512]
                    else:
                        tt_out = box["t"][0:64, qc * 512:(qc + 1) * 512]
                    nc.vector.tensor_mul(
                        tt_out,
                        ctxUs_[hp_][0:64, qc * 512:(qc + 1) * 512],
                        pb_t[0:64, :])
                    if hp_ == 1 and qc == 1:
                        nc.sync.dma_start(
                            ctxT_norm[64:128, p_,
                                      half_ * 1024:(half_ + 1) * 1024],
                            box["t"][0:64, :])

                return [lambda j=j: step(j) for j in range(4)]

            norm_steps = []
            for p in range(2):
                for half in range(2):
                    h0 = p * 2
                    ctx0 = ps.tile([65, 1024], f32, tag="ctx", bufs=2,
                                   name=f"ctx0_{p}_{half}")
                    ctx1 = ps.tile([65, 1024], f32, tag="ctx", bufs=2,
                                   name=f"ctx1_{p}_{half}")
                    ctxs = (ctx0, ctx1)

                    def emit_scores(kt):
                        s0 = ps.tile([128, 1024], f32, tag="a",
                                     name=f"s0_{p}_{half}_{kt}")
                        s1 = ps.tile([128, 1024], f32, tag="a",
                                     name=f"s1_{p}_{half}_{kt}")
                        lhsT = Kt_sb[:, p, kt * 128:(kt + 1) * 128]
                        for qc in range(2):
                            q0 = half * 1024 + qc * 512
                            for s_t, qsrc in ((s0, Qt0_sb), (s1, Qt1_sb)):
                                nc.tensor.matmul(
                                    s_t[:, qc * 512:(qc + 1) * 512],
                                    lhsT=lhsT,
                                    rhs=qsrc[:, p, q0:q0 + 512],
                                    start=True, stop=True)
                        return s0, s1

                    def emit_ctx(kt, ets):
                        for hp in range(2):
                            for qc in range(2):
                                nc.tensor.matmul(
                                    ctxs[hp][:, qc * 512:(qc + 1) * 512],
                                    lhsT=v4[:, h0 + hp, kt, :],
                                    rhs=ets[hp][:, qc * 512:(qc + 1) * 512],
                                    start=(kt == 0), stop=(kt == 15))

                    sc_cur = emit_scores(0)
                    prev_ets = None
                    for kt in range(16):
                        ets = []
                        for hp in range(2):
                            et = expp.tile([128, 1024], bf16, tag="et",
                                           name=f"et_{p}_{half}_{kt}_{hp}")
                            nc.scalar.activation(
                                et[:], sc_cur[hp][:], Exp,
                                bias=maskT_sb[:, kt * 4 + h0 + hp:
                                              kt * 4 + h0 + hp + 1],
                                scale=1.0)
                            ets.append(et)
                        if prev_ets is not None:
                            emit_ctx(kt - 1, prev_ets)
                        if kt < 15:
                            sc_cur = emit_scores(kt + 1)
                        prev_ets = ets
                        if norm_steps and 3 <= kt <= 6:
                            norm_steps.pop(0)()
                    emit_ctx(15, prev_ets)

                    # drain: move ctx out of PSUM, compute 1/sums per head
                    ctxUs, rss = [], []
                    for hp in range(2):
                        ctxU = ctxu.tile([65, 1024], f32, tag="cu", bufs=4,
                                         name=f"ctxU_{p}_{half}_{hp}")
                        nc.vector.tensor_copy(ctxU[:], ctxs[hp][:])
                        s128 = rscr.tile([128, 8], f32, tag="sm",
                                         name=f"s128_{p}_{half}_{hp}")
                        nc.sync.dma_start(s128[:], ctxU[64:65, :])
                        r128 = rscr.tile([128, 8], bf16, tag="r128",
                                         name=f"r128_{p}_{half}_{hp}")
                        with nc.allow_low_precision(
                                reason="fp16 rounding for matmul rhs"):
                            nc.vector.reciprocal(r128[:], s128[:])
                        rs_t = rscr.tile([1, 1024], bf16, tag="rs",
                                         name=f"rs_{p}_{half}_{hp}")
                        nc.sync.dma_start(rs_t[:], r128[:])
                        ctxUs.append(ctxU)
                        rss.append(rs_t)
                    norm_steps = make_norm_steps(p, half, ctxUs, rss)
            # ---- output projection ----
            # qt 0-7 only needs q<1024 whose normalize is already done; the
            # final norm steps (q 1024:2048 of pair 1) interleave with them.
            def emit_outproj(qt):
                tag = "a" if qt % 2 == 0 else "ctx"
                po = ps.tile([128, 1024], f32, tag=tag, name=f"po_{qt}")
                for p in range(2):
                    for ec in range(2):
                        nc.tensor.matmul(
                            po[:, ec * 512:(ec + 1) * 512],
                            lhsT=ctxT_norm[:, p, qt * 128:(qt + 1) * 128],
                            rhs=Wo_sb[:, p, ec * 512:(ec + 1) * 512],
                            start=(p == 0), stop=(p == 1))
                ob = outsb.tile([128, 1024], f32, tag="ob", name=f"ob_{qt}")
                if qt % 2 == 0:
                    nc.vector.tensor_copy(ob[:], po[:])
                else:
                    nc.scalar.copy(ob[:], po[:])
                nc.sync.dma_start(out_d[qt * 128:(qt + 1) * 128, :], ob[:])

            for qt in range(8):
                emit_outproj(qt)
                if norm_steps:
                    norm_steps.pop(0)()
            for st_fn in norm_steps:
                st_fn()
            for qt in range(8, 16):
                emit_outproj(qt)

    nc.compile()
    return nc


def get_program():
    if "nc" not in _cache:
        _cache["nc"] = _build_program()
    return _cache["nc"]


def make_in_maps(query, mask, W_qkv, b_qkv, W_out, b_out):
    query = np.asarray(query, dtype=np.float32)
    mask = np.asarray(mask)
    W_qkv = np.asarray(W_qkv, dtype=np.float32)
    b_qkv = np.asarray(b_qkv, dtype=np.float32)
    W_out = np.asarray(W_out, dtype=np.float32)

    W3 = W_qkv.reshape(DIM, N_HEADS, DIM_PER_HEAD, 3)
    b3 = b_qkv.reshape(N_HEADS, DIM_PER_HEAD, 3)
    maskadd = np.where(mask[:, 0, :], np.float32(-30000.0), np.float32(0.0))

    in_maps = []
    for c in range(N_CORES):
        b = c // 4
        h0 = (c % 4) * HEADS_PER_CORE
        hs = slice(h0, h0 + HEADS_PER_CORE)
        bf = np.float16
        Wq_c = np.ascontiguousarray(
            W3[:, hs, :, 0].reshape(DIM, 256) / SCALE).astype(bf)
        Wk_c = np.ascontiguousarray(W3[:, hs, :, 1].reshape(DIM, 256)).astype(bf)
        Wv_c = np.ascontiguousarray(W3[:, hs, :, 2].reshape(DIM, 256)).astype(bf)
        bq_c = (b3[hs, :, 0].reshape(256) / SCALE).astype(np.float32)
        bk_c = b3[hs, :, 1].reshape(256).astype(np.float32)
        bv_c = b3[hs, :, 2].reshape(1, 256).astype(bf)
        bqk_c = np.ascontiguousarray(
            np.stack([bq_c[:128], bq_c[128:], bk_c[:128], bk_c[128:]], axis=1))
        Wo_c = np.ascontiguousarray(W_out[h0 * 64:(h0 + 4) * 64, :]).astype(bf)
        xT_c = np.ascontiguousarray(query[b].T).astype(bf)
        ma_c = maskadd[b * N_HEADS + h0: b * N_HEADS + h0 + 4]  # [4, 2048]
        maskT_c = np.ascontiguousarray(
            ma_c.reshape(4, 16, 128).transpose(2, 1, 0).reshape(128, 64)
        ).astype(np.float32)
        in_maps.append({
            "xT": xT_c, "Wq": Wq_c, "Wk": Wk_c, "Wv": Wv_c, "Wo": Wo_c,
            "bqk": bqk_c, "bv": bv_c, "maskT": maskT_c,
            "ones_d": np.ones((1, 512), dtype=bf),
            "ones_bf": np.ones((128, 64), dtype=bf),
            "zeros_d": np.zeros((64, 4096), dtype=bf),
        })
    return in_maps


def gather_outputs(results, b_out):
    b_out = np.asarray(b_out, dtype=np.float32)
    out = np.zeros((B, S, DIM), dtype=np.float32)
    for c in range(N_CORES):
        out[c // 4] += results[c]["out"]
    out += b_out[None, None, :]
    return out


def kernel(query, mask, W_qkv, b_qkv, W_out, b_out):
    from concourse.bass_utils import run_bass_kernel_spmd

    nc = get_program()
    in_maps = make_in_maps(query, mask, W_qkv, b_qkv, W_out, b_out)
    res = run_bass_kernel_spmd(nc, in_maps, list(range(N_CORES)))
    return gather_outputs(res.results, b_out)


# revision 22
# speedup vs baseline: 2.0096x; 1.0415x over previous
"""Trainium2 Bass kernel for nn_MultiHeadAttention (B=2, S=2048, D=1024, H=16).

Sharding: batch*heads across 8 cores -> each core handles one batch element's
4 heads (core c: b = c//4, heads h0 = (c%4)*4 .. h0+4).

Per-core device program (all matmuls float32r, full-rate on PE):
  1. QKV projection from pre-transposed activations xT [1024, 2048]:
       Qt/Kt produced transposed ([q_dim, s], 2 heads stacked per 128
       partitions), V produced natural ([s, v_dim], 4 heads side by side)
       with an extra ones column (row-sum trick) appended per head.
  2. Attention per head in "scoresT" layout [key, query]: PE computes
       scoresT tiles, ScalarE applies additive mask + exp in one op
       (bias is per-partition = per-key), PE contracts exp-scores with
       Vaug -> unnormalized ctxT [65, q] where row 64 is the softmax sum.
  3. Normalize: reciprocal of sums, broadcast across partitions via a K=1
       ones-outer-product matmul, multiply on VectorE.
  4. Output projection: ctxT pairs (128 head-dims) x W_out rows -> partial
       output [2048, 1024], DMA'd straight from PSUM to DRAM.
Host sums the 4 partial outputs per batch element and adds b_out.
"""

import math

import numpy as np

N_HEADS = 16
DIM = 1024
DIM_PER_HEAD = 64
B = 2
S = 2048
SCALE = math.sqrt(DIM_PER_HEAD)
N_CORES = 8
HEADS_PER_CORE = 4

_cache = {}


def _build_program():
    import concourse.bass as bass
    import concourse.tile as tile
    from concourse import bacc, mybir

    f32 = mybir.dt.float32
    f32r = mybir.dt.float32r
    bf16 = mybir.dt.float16
    Exp = mybir.ActivationFunctionType.Exp

    nc = bacc.Bacc("TRN2", target_bir_lowering=False, debug=False,
                   num_devices=N_CORES)

    xT = nc.dram_tensor("xT", [DIM, S], bf16, kind="ExternalInput").ap()
    Wq = nc.dram_tensor("Wq", [DIM, 256], bf16, kind="ExternalInput").ap()
    Wk = nc.dram_tensor("Wk", [DIM, 256], bf16, kind="ExternalInput").ap()
    Wv = nc.dram_tensor("Wv", [DIM, 256], bf16, kind="ExternalInput").ap()
    Wo = nc.dram_tensor("Wo", [256, DIM], bf16, kind="ExternalInput").ap()
    bqk = nc.dram_tensor("bqk", [128, 4], f32, kind="ExternalInput").ap()
    bv = nc.dram_tensor("bv", [1, 256], bf16, kind="ExternalInput").ap()
    maskT = nc.dram_tensor("maskT", [128, 64], f32, kind="ExternalInput").ap()
    ones_d = nc.dram_tensor("ones_d", [1, 512], bf16, kind="ExternalInput").ap()
    ones_bf = nc.dram_tensor("ones_bf", [128, 64], bf16, kind="ExternalInput").ap()
    zeros_d = nc.dram_tensor("zeros_d", [64, 4096], bf16, kind="ExternalInput").ap()
    out_d = nc.dram_tensor("out", [S, DIM], f32, kind="ExternalOutput").ap()

    with tile.TileContext(nc) as tc:
        with tc.tile_pool(name="const", bufs=1) as cpool, \
             tc.tile_pool(name="wpool", bufs=1) as wpool, \
             tc.tile_pool(name="qkv", bufs=1) as qkvp, \
             tc.tile_pool(name="xsub", bufs=33) as xsub, \
             tc.tile_pool(name="expp", bufs=5) as expp, \
             tc.tile_pool(name="ctxu", bufs=2) as ctxu, \
             tc.tile_pool(name="outsb", bufs=4) as outsb, \
             tc.tile_pool(name="rscr", bufs=2) as rscr, \
             tc.tile_pool(name="ps", bufs=2, space="PSUM") as ps:

            # --- inputs needed first: Qt/Kt weights, bias, activations ---
            bqk_sb = cpool.tile([128, 4], f32)
            nc.sync.dma_start(bqk_sb[:], bqk[:])
            Wq_sb = wpool.tile([128, 8, 256], bf16)
            nc.sync.dma_start(Wq_sb[:], Wq.rearrange("(c p) j -> p c j", p=128))
            Wk_sb = wpool.tile([128, 8, 256], bf16)
            nc.sync.dma_start(Wk_sb[:], Wk.rearrange("(c p) j -> p c j", p=128))

            xts = []
            for sc in range(4):
                for dc in range(8):
                    xt_t = xsub.tile([128, 512], bf16, tag="x",
                                     name=f"x_{sc}_{dc}")
                    nc.sync.dma_start(
                        xt_t[:], xT[dc * 128:(dc + 1) * 128,
                                    sc * 512:(sc + 1) * 512])
                    xts.append(xt_t)

            Qt0_sb = qkvp.tile([128, 2, S], bf16)
            Qt1_sb = qkvp.tile([128, 2, S], bf16)
            nc.sync.dma_start(Qt0_sb[64:128, :, :], zeros_d.rearrange("p (c s) -> p c s", c=2))
            nc.sync.dma_start(Qt1_sb[0:64, :, :], zeros_d.rearrange("p (c s) -> p c s", c=2))
            Kt_sb = qkvp.tile([128, 2, S], bf16)
            Vaug = qkvp.tile([128, 4 * 16 * 65], bf16)
            v4 = Vaug.rearrange("p (h k j) -> p h k j", h=4, k=16)
            ctxT_norm = qkvp.tile([128, 2, S], bf16)

            # ---- Q/K projection (transposed layout, 2 heads per 128 rows) ----
            for sc in range(4):
                for qk in range(2):
                    W = (Wq_sb, Wk_sb)[qk]
                    for p in range(2):
                        ps_t = ps.tile([128, 512], f32,
                                       tag="a" if (qk * 2 + p) % 2 == 0 else "ctx",
                                       name=f"pq_{sc}_{qk}_{p}")
                        for dc in range(8):
                            nc.tensor.matmul(
                                ps_t[:], lhsT=W[:, dc, p * 128:(p + 1) * 128],
                                rhs=xts[sc * 8 + dc][:],
                                start=(dc == 0), stop=(dc == 7))
                        ssl = slice(sc * 512, (sc + 1) * 512)
                        bias = bqk_sb[:, 2 * qk + p: 2 * qk + p + 1]
                        if qk == 1:
                            nc.vector.tensor_scalar_add(
                                Kt_sb[:, p, ssl], ps_t[:], bias)
                        else:
                            nc.vector.tensor_scalar_add(
                                Qt0_sb[0:64, p, ssl], ps_t[0:64, :],
                                bias[0:64, :])
                            nc.vector.tensor_scalar_add(
                                Qt1_sb[64:128, p, ssl], ps_t[64:128, :],
                                bias[64:128, :])

            # --- later inputs (DMAs overlap the Q/K projection above) ---
            Wv_sb = wpool.tile([128, 8, 256], bf16)
            nc.sync.dma_start(Wv_sb[:], Wv.rearrange("(c p) j -> p c j", p=128))
            bv_sb = cpool.tile([1, 256], bf16)
            nc.sync.dma_start(bv_sb[:], bv[:])
            ones = cpool.tile([1, 512], bf16)
            nc.sync.dma_start(ones[:], ones_d[:])
            nc.sync.dma_start(v4[:, :, :, 64],
                              ones_bf[:, :].rearrange("p (h k) -> p h k", h=4))
            maskT_sb = cpool.tile([128, 64], f32)
            nc.sync.dma_start(maskT_sb[:], maskT[:])
            Wo_sb = wpool.tile([128, 2, 1024], bf16)
            nc.sync.dma_start(Wo_sb[:], Wo.rearrange("(c p) e -> p c e", p=128))

            # ---- V projection (natural layout, all 4 heads side by side) ----
            for st in range(16):
                sc, stl = st // 4, st % 4
                pv_t = ps.tile([128, 256], f32,
                               tag="a" if st % 2 == 0 else "ctx",
                               name=f"pv_{st}")
                for dc in range(8):
                    nc.tensor.matmul(
                        pv_t[:],
                        lhsT=xts[sc * 8 + dc][:, stl * 128:(stl + 1) * 128],
                        rhs=Wv_sb[:, dc, :], start=(dc == 0), stop=False)
                nc.tensor.matmul(pv_t[:], lhsT=ones[0:1, 0:128], rhs=bv_sb[:],
                                 start=False, stop=True)
                nc.vector.tensor_copy(
                    v4[:, :, st, 0:64],
                    pv_t[:].rearrange("p (h j) -> p h j", h=4))

            # ---- attention, one head PAIR at a time ----
            # scoresT layout [key, query]; the two heads of a pair live at
            # partition bases 0 and 64, so their K=64 score matmuls row-pack
            # and run concurrently on the PE (2x). q is processed in halves of
            # 1024 so both heads' ctx accumulators fit in PSUM. The ctx
            # matmuls for step kt are issued while ScalarE still exps step
            # kt+1 (one-step lag) so the PE never stalls on ACT. The
            # normalize work for a block is deferred into the next block.
            def make_norm_steps(p_, half_, ctxUs_, rss_):
                box = {}

                def step(j):
                    hp_, qc = j // 2, j % 2
                    if hp_ == 1 and "t" not in box:
                        box["t"] = ctxu.tile([64, 1024], bf16, tag="cn",
                                             bufs=2, name=f"ctxN_{p_}_{half_}")
                    pb_t = ps.tile([128, 512], f32, tag="a",
                                   name=f"pb_{p_}_{half_}_{j}")
                    nc.tensor.matmul(pb_t[:], lhsT=ones[0:1, 0:128],
                                     rhs=rss_[hp_][0:1, qc * 512:(qc + 1) * 512],
                                     start=True, stop=True)
                    if hp_ == 0:
                        tt_out = ctxT_norm[0:64, p_,
                                           half_ * 1024 + qc * 512:
                                           half_ * 1024 + (qc + 1) * 512]
                    else:
                        tt_out = box["t"][0:64, qc * 512:(qc + 1) * 512]
                    nc.vector.tensor_mul(
                        tt_out,
                        ctxUs_[hp_][0:64, qc * 512:(qc + 1) * 512],
                        pb_t[0:64, :])
                    if hp_ == 1 and qc == 1:
                        nc.sync.dma_start(
                            ctxT_norm[64:128, p_,
                                      half_ * 1024:(half_ + 1) * 1024],
                            box["t"][0:64, :])

                return [lambda j=j: step(j) for j in range(4)]

            norm_steps = []
            for p in range(2):
                for half in range(2):
                    h0 = p * 2
                    ctx0 = ps.tile([65, 1024], f32, tag="ctx", bufs=2,
                                   name=f"ctx0_{p}_{half}")
                    ctx1 = ps.tile([65, 1024], f32, tag="ctx", bufs=2,
                                   name=f"ctx1_{p}_{half}")
                    ctxs = (ctx0, ctx1)

                    def emit_scores(kt):
                        s0 = ps.tile([128, 1024], f32, tag="a",
                                     name=f"s0_{p}_{half}_{kt}")
                        s1 = ps.tile([128, 1024], f32, tag="a",
                                     name=f"s1_{p}_{half}_{kt}")
                        lhsT = Kt_sb[:, p, kt * 128:(kt + 1) * 128]
                        for qc in range(2):
                            q0 = half * 1024 + qc * 512
                            for s_t, qsrc in ((s0, Qt0_sb), (s1, Qt1_sb)):
                                nc.tensor.matmul(
                                    s_t[:, qc * 512:(qc + 1) * 512],
                                    lhsT=lhsT,
                                    rhs=qsrc[:, p, q0:q0 + 512],
                                    start=True, stop=True)
                        return s0, s1

                    def emit_ctx(kt, ets):
                        for hp in range(2):
                            for qc in range(2):
                                nc.tensor.matmul(
                                    ctxs[hp][:, qc * 512:(qc + 1) * 512],
                                    lhsT=v4[:, h0 + hp, kt, :],
                                    rhs=ets[hp][:, qc * 512:(qc + 1) * 512],
                                    start=(kt == 0), stop=(kt == 15))

                    sc_cur = emit_scores(0)
                    prev_ets = None
                    for kt in range(16):
                        ets = []
                        for hp in range(2):
                            et = expp.tile([128, 1024], bf16, tag="et",
                                           name=f"et_{p}_{half}_{kt}_{hp}")
                            nc.scalar.activation(
                                et[:], sc_cur[hp][:], Exp,
                                bias=maskT_sb[:, kt * 4 + h0 + hp:
                                              kt * 4 + h0 + hp + 1],
                                scale=1.0)
                            ets.append(et)
                        if prev_ets is not None:
                            emit_ctx(kt - 1, prev_ets)
                        if kt < 15:
                            sc_cur = emit_scores(kt + 1)
                        prev_ets = ets
                        if norm_steps and 3 <= kt <= 6:
                            norm_steps.pop(0)()
                    emit_ctx(15, prev_ets)

                    # drain: move ctx out of PSUM, compute 1/sums per head
                    ctxUs, rss = [], []
                    for hp in range(2):
                        ctxU = ctxu.tile([65, 1024], f32, tag="cu", bufs=4,
                                         name=f"ctxU_{p}_{half}_{hp}")
                        nc.vector.tensor_copy(ctxU[:], ctxs[hp][:])
                        s128 = rscr.tile([128, 8], f32, tag="sm",
                                         name=f"s128_{p}_{half}_{hp}")
                        nc.sync.dma_start(s128[:], ctxU[64:65, :])
                        r128 = rscr.tile([128, 8], bf16, tag="r128",
                                         name=f"r128_{p}_{half}_{hp}")
                        with nc.allow_low_precision(
                                reason="fp16 rounding for matmul rhs"):
                            nc.vector.reciprocal(r128[:], s128[:])
                        rs_t = rscr.tile([1, 1024], bf16, tag="rs",
                                         name=f"rs_{p}_{half}_{hp}")
                        nc.sync.dma_start(rs_t[:], r128[:])
                        ctxUs.append(ctxU)
                        rss.append(rs_t)
                    norm_steps = make_norm_steps(p, half, ctxUs, rss)
            # ---- output projection ----
            # qt 0-7 only needs q<1024 whose normalize is already done; the
            # final norm steps (q 1024:2048 of pair 1) interleave with them.
            def emit_outproj(qt):
                tag = "a" if qt % 2 == 0 else "ctx"
                po = ps.tile([128, 1024], f32, tag=tag, name=f"po_{qt}")
                for p in range(2):
                    for ec in range(2):
                        nc.tensor.matmul(
                            po[:, ec * 512:(ec + 1) * 512],
                            lhsT=ctxT_norm[:, p, qt * 128:(qt + 1) * 128],
                            rhs=Wo_sb[:, p, ec * 512:(ec + 1) * 512],
                            start=(p == 0), stop=(p == 1))
                ob = outsb.tile([128, 1024], f32, tag="ob", name=f"ob_{qt}")
                if qt % 2 == 0:
                    nc.vector.tensor_copy(ob[:], po[:])
                else:
                    nc.scalar.copy(ob[:], po[:])
                nc.sync.dma_start(out_d[qt * 128:(qt + 1) * 128, :], ob[:])

            for qt in range(8):
                emit_outproj(qt)
                if norm_steps:
                    norm_steps.pop(0)()
            for st_fn in norm_steps:
                st_fn()
            for qt in range(8, 16):
                emit_outproj(qt)

    nc.compile()
    return nc


def get_program():
    if "nc" not in _cache:
        _cache["nc"] = _build_program()
    return _cache["nc"]


def make_in_maps(query, mask, W_qkv, b_qkv, W_out, b_out):
    query = np.asarray(query, dtype=np.float32)
    mask = np.asarray(mask)
    W_qkv = np.asarray(W_qkv, dtype=np.float32)
    b_qkv = np.asarray(b_qkv, dtype=np.float32)
    W_out = np.asarray(W_out, dtype=np.float32)

    W3 = W_qkv.reshape(DIM, N_HEADS, DIM_PER_HEAD, 3)
    b3 = b_qkv.reshape(N_HEADS, DIM_PER_HEAD, 3)
    maskadd = np.where(mask[:, 0, :], np.float32(-30000.0), np.float32(0.0))

    in_maps = []
    for c in range(N_CORES):
        b = c // 4
        h0 = (c % 4) * HEADS_PER_CORE
        hs = slice(h0, h0 + HEADS_PER_CORE)
        bf = np.float16
        Wq_c = np.ascontiguousarray(
            W3[:, hs, :, 0].reshape(DIM, 256) / SCALE).astype(bf)
        Wk_c = np.ascontiguousarray(W3[:, hs, :, 1].reshape(DIM, 256)).astype(bf)
        Wv_c = np.ascontiguousarray(W3[:, hs, :, 2].reshape(DIM, 256)).astype(bf)
        bq_c = (b3[hs, :, 0].reshape(256) / SCALE).astype(np.float32)
        bk_c = b3[hs, :, 1].reshape(256).astype(np.float32)
        bv_c = b3[hs, :, 2].reshape(1, 256).astype(bf)
        bqk_c = np.ascontiguousarray(
            np.stack([bq_c[:128], bq_c[128:], bk_c[:128], bk_c[128:]], axis=1))
        Wo_c = np.ascontiguousarray(W_out[h0 * 64:(h0 + 4) * 64, :]).astype(bf)
        xT_c = np.ascontiguousarray(query[b].T).astype(bf)
        ma_c = maskadd[b * N_HEADS + h0: b * N_HEADS + h0 + 4]  # [4, 2048]
        maskT_c = np.ascontiguousarray(
            ma_c.reshape(4, 16, 128).transpose(2, 1, 0).reshape(128, 64)
        ).astype(np.float32)
        in_maps.append({
            "xT": xT_c, "Wq": Wq_c, "Wk": Wk_c, "Wv": Wv_c, "Wo": Wo_c,
            "bqk": bqk_c, "bv": bv_c, "maskT": maskT_c,
            "ones_d": np.ones((1, 512), dtype=bf),
            "ones_bf": np.ones((128, 64), dtype=bf),
            "zeros_d": np.zeros((64, 4096), dtype=bf),
        })
    return in_maps


def gather_outputs(results, b_out):
    b_out = np.asarray(b_out, dtype=np.float32)
    out = np.zeros((B, S, DIM), dtype=np.float32)
    for c in range(N_CORES):
        out[c // 4] += results[c]["out"]
    out += b_out[None, None, :]
    return out


def kernel(query, mask, W_qkv, b_qkv, W_out, b_out):
    from concourse.bass_utils import run_bass_kernel_spmd

    nc = get_program()
    in_maps = make_in_maps(query, mask, W_qkv, b_qkv, W_out, b_out)
    res = run_bass_kernel_spmd(nc, in_maps, list(range(N_CORES)))
    return gather_outputs(res.results, b_out)


# revision 23
# speedup vs baseline: 2.0273x; 1.0088x over previous
"""Trainium2 Bass kernel for nn_MultiHeadAttention (B=2, S=2048, D=1024, H=16).

Sharding: batch*heads across 8 cores -> each core handles one batch element's
4 heads (core c: b = c//4, heads h0 = (c%4)*4 .. h0+4).

Per-core device program (all matmuls float32r, full-rate on PE):
  1. QKV projection from pre-transposed activations xT [1024, 2048]:
       Qt/Kt produced transposed ([q_dim, s], 2 heads stacked per 128
       partitions), V produced natural ([s, v_dim], 4 heads side by side)
       with an extra ones column (row-sum trick) appended per head.
  2. Attention per head in "scoresT" layout [key, query]: PE computes
       scoresT tiles, ScalarE applies additive mask + exp in one op
       (bias is per-partition = per-key), PE contracts exp-scores with
       Vaug -> unnormalized ctxT [65, q] where row 64 is the softmax sum.
  3. Normalize: reciprocal of sums, broadcast across partitions via a K=1
       ones-outer-product matmul, multiply on VectorE.
  4. Output projection: ctxT pairs (128 head-dims) x W_out rows -> partial
       output [2048, 1024], DMA'd straight from PSUM to DRAM.
Host sums the 4 partial outputs per batch element and adds b_out.
"""

import math

import numpy as np

N_HEADS = 16
DIM = 1024
DIM_PER_HEAD = 64
B = 2
S = 2048
SCALE = math.sqrt(DIM_PER_HEAD)
N_CORES = 8
HEADS_PER_CORE = 4

_cache = {}


def _build_program():
    import concourse.bass as bass
    import concourse.tile as tile
    from concourse import bacc, mybir

    f32 = mybir.dt.float32
    f32r = mybir.dt.float32r
    bf16 = mybir.dt.float16
    Exp = mybir.ActivationFunctionType.Exp

    nc = bacc.Bacc("TRN2", target_bir_lowering=False, debug=False,
                   num_devices=N_CORES)

    xT = nc.dram_tensor("xT", [DIM, S], bf16, kind="ExternalInput").ap()
    Wq = nc.dram_tensor("Wq", [DIM, 256], bf16, kind="ExternalInput").ap()
    Wk = nc.dram_tensor("Wk", [DIM, 256], bf16, kind="ExternalInput").ap()
    Wv = nc.dram_tensor("Wv", [DIM, 256], bf16, kind="ExternalInput").ap()
    Wo = nc.dram_tensor("Wo", [256, DIM], bf16, kind="ExternalInput").ap()
    bqk = nc.dram_tensor("bqk", [128, 4], f32, kind="ExternalInput").ap()
    bv = nc.dram_tensor("bv", [1, 256], bf16, kind="ExternalInput").ap()
    maskT = nc.dram_tensor("maskT", [128, 64], f32, kind="ExternalInput").ap()
    ones_d = nc.dram_tensor("ones_d", [1, 512], bf16, kind="ExternalInput").ap()
    ones_bf = nc.dram_tensor("ones_bf", [128, 64], bf16, kind="ExternalInput").ap()
    zeros_d = nc.dram_tensor("zeros_d", [64, 4096], bf16, kind="ExternalInput").ap()
    out_d = nc.dram_tensor("out", [S, DIM], f32, kind="ExternalOutput").ap()

    with tile.TileContext(nc) as tc:
        with tc.tile_pool(name="const", bufs=1) as cpool, \
             tc.tile_pool(name="wpool", bufs=1) as wpool, \
             tc.tile_pool(name="qkv", bufs=1) as qkvp, \
             tc.tile_pool(name="xsub", bufs=33) as xsub, \
             tc.tile_pool(name="expp", bufs=5) as expp, \
             tc.tile_pool(name="ctxu", bufs=2) as ctxu, \
             tc.tile_pool(name="outsb", bufs=4) as outsb, \
             tc.tile_pool(name="rscr", bufs=2) as rscr, \
             tc.tile_pool(name="ps", bufs=2, space="PSUM") as ps:

            # --- inputs needed first: Qt/Kt weights, bias, activations ---
            bqk_sb = cpool.tile([128, 4], f32)
            nc.sync.dma_start(bqk_sb[:], bqk[:])
            Wq_sb = wpool.tile([128, 8, 256], bf16)
            nc.sync.dma_start(Wq_sb[:], Wq.rearrange("(c p) j -> p c j", p=128))

            xts = []

            def load_xts(sc):
                for dc in range(8):
                    xt_t = xsub.tile([128, 512], bf16, tag="x",
                                     name=f"x_{sc}_{dc}")
                    nc.sync.dma_start(
                        xt_t[:], xT[dc * 128:(dc + 1) * 128,
                                    sc * 512:(sc + 1) * 512])
                    xts.append(xt_t)

            load_xts(0)
            Wk_sb = wpool.tile([128, 8, 256], bf16)
            nc.sync.dma_start(Wk_sb[:], Wk.rearrange("(c p) j -> p c j", p=128))
            for _sc in range(1, 4):
                load_xts(_sc)

            Qt0_sb = qkvp.tile([128, 2, S], bf16)
            Qt1_sb = qkvp.tile([128, 2, S], bf16)
            nc.sync.dma_start(Qt0_sb[64:128, :, :], zeros_d.rearrange("p (c s) -> p c s", c=2))
            nc.sync.dma_start(Qt1_sb[0:64, :, :], zeros_d.rearrange("p (c s) -> p c s", c=2))
            Kt_sb = qkvp.tile([128, 2, S], bf16)
            Vaug = qkvp.tile([128, 4 * 16 * 65], bf16)
            v4 = Vaug.rearrange("p (h k j) -> p h k j", h=4, k=16)
            ctxT_norm = qkvp.tile([128, 2, S], bf16)

            # ---- Q/K projection (transposed layout, 2 heads per 128 rows) ----
            for sc in range(4):
                for qk in range(2):
                    W = (Wq_sb, Wk_sb)[qk]
                    for p in range(2):
                        ps_t = ps.tile([128, 512], f32,
                                       tag="a" if (qk * 2 + p) % 2 == 0 else "ctx",
                                       name=f"pq_{sc}_{qk}_{p}")
                        for dc in range(8):
                            nc.tensor.matmul(
                                ps_t[:], lhsT=W[:, dc, p * 128:(p + 1) * 128],
                                rhs=xts[sc * 8 + dc][:],
                                start=(dc == 0), stop=(dc == 7))
                        ssl = slice(sc * 512, (sc + 1) * 512)
                        bias = bqk_sb[:, 2 * qk + p: 2 * qk + p + 1]
                        if qk == 1:
                            nc.vector.tensor_scalar_add(
                                Kt_sb[:, p, ssl], ps_t[:], bias)
                        else:
                            nc.vector.tensor_scalar_add(
                                Qt0_sb[0:64, p, ssl], ps_t[0:64, :],
                                bias[0:64, :])
                            nc.vector.tensor_scalar_add(
                                Qt1_sb[64:128, p, ssl], ps_t[64:128, :],
                                bias[64:128, :])

            # --- later inputs (DMAs overlap the Q/K projection above) ---
            Wv_sb = wpool.tile([128, 8, 256], bf16)
            nc.sync.dma_start(Wv_sb[:], Wv.rearrange("(c p) j -> p c j", p=128))
            bv_sb = cpool.tile([1, 256], bf16)
            nc.sync.dma_start(bv_sb[:], bv[:])
            ones = cpool.tile([1, 512], bf16)
            nc.sync.dma_start(ones[:], ones_d[:])
            nc.sync.dma_start(v4[:, :, :, 64],
                              ones_bf[:, :].rearrange("p (h k) -> p h k", h=4))
            maskT_sb = cpool.tile([128, 64], f32)
            nc.sync.dma_start(maskT_sb[:], maskT[:])
            Wo_sb = wpool.tile([128, 2, 1024], bf16)
            nc.sync.dma_start(Wo_sb[:], Wo.rearrange("(c p) e -> p c e", p=128))

            # ---- V projection (natural layout, all 4 heads side by side) ----
            for st in range(16):
                sc, stl = st // 4, st % 4
                pv_t = ps.tile([128, 256], f32,
                               tag="a" if st % 2 == 0 else "ctx",
                               name=f"pv_{st}")
                for dc in range(8):
                    nc.tensor.matmul(
                        pv_t[:],
                        lhsT=xts[sc * 8 + dc][:, stl * 128:(stl + 1) * 128],
                        rhs=Wv_sb[:, dc, :], start=(dc == 0), stop=False)
                nc.tensor.matmul(pv_t[:], lhsT=ones[0:1, 0:128], rhs=bv_sb[:],
                                 start=False, stop=True)
                nc.vector.tensor_copy(
                    v4[:, :, st, 0:64],
                    pv_t[:].rearrange("p (h j) -> p h j", h=4))

            # ---- attention, one head PAIR at a time ----
            # scoresT layout [key, query]; the two heads of a pair live at
            # partition bases 0 and 64, so their K=64 score matmuls row-pack
            # and run concurrently on the PE (2x). q is processed in halves of
            # 1024 so both heads' ctx accumulators fit in PSUM. The ctx
            # matmuls for step kt are issued while ScalarE still exps step
            # kt+1 (one-step lag) so the PE never stalls on ACT. The
            # normalize work for a block is deferred into the next block.
            def make_norm_steps(p_, half_, ctxUs_, rss_):
                box = {}

                def step(j):
                    hp_, qc = j // 2, j % 2
                    if hp_ == 1 and "t" not in box:
                        box["t"] = ctxu.tile([64, 1024], bf16, tag="cn",
                                             bufs=2, name=f"ctxN_{p_}_{half_}")
                    pb_t = ps.tile([128, 512], f32, tag="a",
                                   name=f"pb_{p_}_{half_}_{j}")
                    nc.tensor.matmul(pb_t[:], lhsT=ones[0:1, 0:128],
                                     rhs=rss_[hp_][0:1, qc * 512:(qc + 1) * 512],
                                     start=True, stop=True)
                    if hp_ == 0:
                        tt_out = ctxT_norm[0:64, p_,
                                           half_ * 1024 + qc * 512:
                                           half_ * 1024 + (qc + 1) * 512]
                    else:
                        tt_out = box["t"][0:64, qc * 512:(qc + 1) * 512]
                    nc.vector.tensor_mul(
                        tt_out,
                        ctxUs_[hp_][0:64, qc * 512:(qc + 1) * 512],
                        pb_t[0:64, :])
                    if hp_ == 1 and qc == 1:
                        nc.sync.dma_start(
                            ctxT_norm[64:128, p_,
                                      half_ * 1024:(half_ + 1) * 1024],
                            box["t"][0:64, :])

                return [lambda j=j: step(j) for j in range(4)]

            norm_steps = []
            for p in range(2):
                for half in range(2):
                    h0 = p * 2
                    ctx0 = ps.tile([65, 1024], f32, tag="ctx", bufs=2,
                                   name=f"ctx0_{p}_{half}")
                    ctx1 = ps.tile([65, 1024], f32, tag="ctx", bufs=2,
                                   name=f"ctx1_{p}_{half}")
                    ctxs = (ctx0, ctx1)

                    def emit_scores(kt):
                        s0 = ps.tile([128, 1024], f32, tag="a",
                                     name=f"s0_{p}_{half}_{kt}")
                        s1 = ps.tile([128, 1024], f32, tag="a",
                                     name=f"s1_{p}_{half}_{kt}")
                        lhsT = Kt_sb[:, p, kt * 128:(kt + 1) * 128]
                        for qc in range(2):
                            q0 = half * 1024 + qc * 512
                            for s_t, qsrc in ((s0, Qt0_sb), (s1, Qt1_sb)):
                                nc.tensor.matmul(
                                    s_t[:, qc * 512:(qc + 1) * 512],
                                    lhsT=lhsT,
                                    rhs=qsrc[:, p, q0:q0 + 512],
                                    start=True, stop=True)
                        return s0, s1

                    def emit_ctx(kt, ets):
                        for hp in range(2):
                            for qc in range(2):
                                nc.tensor.matmul(
                                    ctxs[hp][:, qc * 512:(qc + 1) * 512],
                                    lhsT=v4[:, h0 + hp, kt, :],
                                    rhs=ets[hp][:, qc * 512:(qc + 1) * 512],
                                    start=(kt == 0), stop=(kt == 15))

                    sc_cur = emit_scores(0)
                    prev_ets = None
                    for kt in range(16):
                        ets = []
                        for hp in range(2):
                            et = expp.tile([128, 1024], bf16, tag="et",
                                           name=f"et_{p}_{half}_{kt}_{hp}")
                            nc.scalar.activation(
                                et[:], sc_cur[hp][:], Exp,
                                bias=maskT_sb[:, kt * 4 + h0 + hp:
                                              kt * 4 + h0 + hp + 1],
                                scale=1.0)
                            ets.append(et)
                        if prev_ets is not None:
                            emit_ctx(kt - 1, prev_ets)
                        if kt < 15:
                            sc_cur = emit_scores(kt + 1)
                        prev_ets = ets
                        if norm_steps and 3 <= kt <= 6:
                            norm_steps.pop(0)()
                    emit_ctx(15, prev_ets)

                    # drain: move ctx out of PSUM, compute 1/sums per head
                    ctxUs, rss = [], []
                    for hp in range(2):
                        ctxU = ctxu.tile([65, 1024], f32, tag="cu", bufs=4,
                                         name=f"ctxU_{p}_{half}_{hp}")
                        nc.vector.tensor_copy(ctxU[:], ctxs[hp][:])
                        s128 = rscr.tile([128, 8], f32, tag="sm",
                                         name=f"s128_{p}_{half}_{hp}")
                        nc.sync.dma_start(s128[:], ctxU[64:65, :])
                        r128 = rscr.tile([128, 8], bf16, tag="r128",
                                         name=f"r128_{p}_{half}_{hp}")
                        with nc.allow_low_precision(
                                reason="fp16 rounding for matmul rhs"):
                            nc.vector.reciprocal(r128[:], s128[:])
                        rs_t = rscr.tile([1, 1024], bf16, tag="rs",
                                         name=f"rs_{p}_{half}_{hp}")
                        nc.sync.dma_start(rs_t[:], r128[:])
                        ctxUs.append(ctxU)
                        rss.append(rs_t)
                    norm_steps = make_norm_steps(p, half, ctxUs, rss)
            # ---- output projection ----
            # qt 0-7 only needs q<1024 whose normalize is already done; the
            # final norm steps (q 1024:2048 of pair 1) interleave with them.
            def emit_outproj(qt):
                tag = "a" if qt % 2 == 0 else "ctx"
                po = ps.tile([128, 1024], f32, tag=tag, name=f"po_{qt}")
                for p in range(2):
                    for ec in range(2):
                        nc.tensor.matmul(
                            po[:, ec * 512:(ec + 1) * 512],
                            lhsT=ctxT_norm[:, p, qt * 128:(qt + 1) * 128],
                            rhs=Wo_sb[:, p, ec * 512:(ec + 1) * 512],
                            start=(p == 0), stop=(p == 1))
                ob = outsb.tile([128, 1024], f32, tag="ob", name=f"ob_{qt}")
                if qt % 2 == 0:
                    nc.vector.tensor_copy(ob[:], po[:])
                else:
                    nc.scalar.copy(ob[:], po[:])
                nc.sync.dma_start(out_d[qt * 128:(qt + 1) * 128, :], ob[:])

            for qt in range(8):
                emit_outproj(qt)
                if norm_steps:
                    norm_steps.pop(0)()
            for st_fn in norm_steps:
                st_fn()
            for qt in range(8, 16):
                emit_outproj(qt)

    nc.compile()
    return nc


def get_program():
    if "nc" not in _cache:
        _cache["nc"] = _build_program()
    return _cache["nc"]


def make_in_maps(query, mask, W_qkv, b_qkv, W_out, b_out):
    query = np.asarray(query, dtype=np.float32)
    mask = np.asarray(mask)
    W_qkv = np.asarray(W_qkv, dtype=np.float32)
    b_qkv = np.asarray(b_qkv, dtype=np.float32)
    W_out = np.asarray(W_out, dtype=np.float32)

    W3 = W_qkv.reshape(DIM, N_HEADS, DIM_PER_HEAD, 3)
    b3 = b_qkv.reshape(N_HEADS, DIM_PER_HEAD, 3)
    maskadd = np.where(mask[:, 0, :], np.float32(-30000.0), np.float32(0.0))

    in_maps = []
    for c in range(N_CORES):
        b = c // 4
        h0 = (c % 4) * HEADS_PER_CORE
        hs = slice(h0, h0 + HEADS_PER_CORE)
        bf = np.float16
        Wq_c = np.ascontiguousarray(
            W3[:, hs, :, 0].reshape(DIM, 256) / SCALE).astype(bf)
        Wk_c = np.ascontiguousarray(W3[:, hs, :, 1].reshape(DIM, 256)).astype(bf)
        Wv_c = np.ascontiguousarray(W3[:, hs, :, 2].reshape(DIM, 256)).astype(bf)
        bq_c = (b3[hs, :, 0].reshape(256) / SCALE).astype(np.float32)
        bk_c = b3[hs, :, 1].reshape(256).astype(np.float32)
        bv_c = b3[hs, :, 2].reshape(1, 256).astype(bf)
        bqk_c = np.ascontiguousarray(
            np.stack([bq_c[:128], bq_c[128:], bk_c[:128], bk_c[128:]], axis=1))
        Wo_c = np.ascontiguousarray(W_out[h0 * 64:(h0 + 4) * 64, :]).astype(bf)
        xT_c = np.ascontiguousarray(query[b].T).astype(bf)
        ma_c = maskadd[b * N_HEADS + h0: b * N_HEADS + h0 + 4]  # [4, 2048]
        maskT_c = np.ascontiguousarray(
            ma_c.reshape(4, 16, 128).transpose(2, 1, 0).reshape(128, 64)
        ).astype(np.float32)
        in_maps.append({
            "xT": xT_c, "Wq": Wq_c, "Wk": Wk_c, "Wv": Wv_c, "Wo": Wo_c,
            "bqk": bqk_c, "bv": bv_c, "maskT": maskT_c,
            "ones_d": np.ones((1, 512), dtype=bf),
            "ones_bf": np.ones((128, 64), dtype=bf),
            "zeros_d": np.zeros((64, 4096), dtype=bf),
        })
    return in_maps


def gather_outputs(results, b_out):
    b_out = np.asarray(b_out, dtype=np.float32)
    out = np.zeros((B, S, DIM), dtype=np.float32)
    for c in range(N_CORES):
        out[c // 4] += results[c]["out"]
    out += b_out[None, None, :]
    return out


def kernel(query, mask, W_qkv, b_qkv, W_out, b_out):
    from concourse.bass_utils import run_bass_kernel_spmd

    nc = get_program()
    in_maps = make_in_maps(query, mask, W_qkv, b_qkv, W_out, b_out)
    res = run_bass_kernel_spmd(nc, in_maps, list(range(N_CORES)))
    return gather_outputs(res.results, b_out)
